# revision 1
# baseline (speedup 1.0000x reference)
"""Trainium2 Bass kernel for nn_Attention_42288247996512 (sparse causal cross-attention).

reference:
  q = x @ Wq.T; k = cross @ Wk.T; v = x @ Wv.T
  logits = q @ k.T  (causal mask; padding mask m_q*m_k + eye > 0)
  out = softmax(logits / sqrt(128)) @ v

Sharding: 8 cores = 4 batches x 2 query-strips. Each strip is 8 query blocks
(128 rows) chosen so both strips have identical causal-chunk structure
(SPMD: one program, per-core data). Host pre-transposes inputs (avoids
on-chip fp32 transposes), pre-scales Wq by 1/sqrt(128), builds additive
mask tiles, and does the final denominator divide + scatter.

On-chip per core: kT/qT/v projections (float32r matmuls), then per
block-pair: logits -> +mask (DVE) -> exp (ACT, per-partition q-mask bias,
accum_out denominator) -> PE transpose -> AV matmul -> store out.T.
"""
import math
import os
import threading

import ml_dtypes
import numpy as np

B, S, D, DA = 4, 2048, 1024, 128
P = 128
NCORES = 8
BIG = 32768.0  # power of two: exactly representable in bf16
NBLK = S // P  # 16 key blocks / query blocks per batch
NQ = 1024      # query rows per core strip

# strips: pairs of adjacent blocks, same chunk-count multiset on both strips
STRIPS = [
    [0, 1, 14, 15, 6, 7, 8, 9],
    [2, 3, 12, 13, 4, 5, 10, 11],
]
PAIR_C = [1, 4, 2, 3]  # 512-wide key chunks per pair (same for both strips)

_BUILD_LOCK = threading.Lock()
_CACHE: dict = {}


def _build():
    from contextlib import ExitStack

    import concourse.bass as bass
    import concourse.mybir as mybir
    import concourse.tile as tile
    from concourse import bacc
    from concourse.masks import make_identity

    dt = mybir.dt
    f32 = dt.float32
    f32r = dt.float32r
    AF = mybir.ActivationFunctionType
    ALU = mybir.AluOpType

    nc = bacc.Bacc("TRN2", target_bir_lowering=False, debug=False)

    bf16 = dt.bfloat16
    xT = nc.dram_tensor("xT", [D, S], f32r, kind="ExternalInput").ap()
    cT = nc.dram_tensor("cT", [D, S], f32r, kind="ExternalInput").ap()
    xqT = nc.dram_tensor("xqT", [D, NQ], f32r, kind="ExternalInput").ap()
    wqT = nc.dram_tensor("wqT", [D, DA], f32r, kind="ExternalInput").ap()
    wkT = nc.dram_tensor("wkT", [D, DA], f32r, kind="ExternalInput").ap()
    wvT = nc.dram_tensor("wvT", [D, D], f32r, kind="ExternalInput").ap()
    # additive masks in bf16 (values are sums of +-2^15/2^16: exact in bf16)
    kmb = nc.dram_tensor("kmb", [P, 1536], bf16, kind="ExternalInput").ap()
    qmn = nc.dram_tensor("qmn", [P, 8], f32, kind="ExternalInput").ap()
    dmask = nc.dram_tensor("dmask", [8, P, 512], bf16, kind="ExternalInput").ap()

    outT = nc.dram_tensor("outT", [D, NQ], f32, kind="ExternalOutput").ap()
    den = nc.dram_tensor("den", [P, 8], f32, kind="ExternalOutput").ap()

    KC = D // P  # 8 contraction chunks for projections

    with tile.TileContext(nc) as tc, ExitStack() as ctx:
        const = ctx.enter_context(tc.tile_pool(name="const", bufs=1))
        persist = ctx.enter_context(tc.tile_pool(name="persist", bufs=1))
        stream = ctx.enter_context(tc.tile_pool(name="stream", bufs=2))

        # ---- constants / weights / masks ----
        ident_f32 = const.tile([P, P], f32, name="ident_f32")
        make_identity(nc, ident_f32)
        ident = const.tile([P, P], f32r, name="ident")
        nc.vector.tensor_copy(ident[:], ident_f32[:])

        # All input DMAs go on the single SP HWDGE queue: one InstDMACopy
        # spreads over all 16 SDMA slots (full ~358GB/s), and the strict
        # FIFO gives exact control of delivery order = consumption order.
        # Tiles are declared here; their loads are emitted at the point in
        # the phase schedule where the FIFO should deliver them.
        wvT_r = wvT.rearrange("(kc p) m -> kc p m", p=P)
        wq_sb = const.tile([P, KC, DA], f32r, name="wq_sb")
        wk_sb = const.tile([P, KC, DA], f32r, name="wk_sb")
        kmb_sb = const.tile([P, 1536], bf16, name="kmb_sb")
        qmn_sb = const.tile([P, 8], f32, name="qmn_sb")
        dm_sb = const.tile([P, 8, 512], bf16, name="dm_sb")

        kT_sb = persist.tile([P, S], f32r, name="kT_sb")
        qT_sb = persist.tile([P, NQ], f32r, name="qT_sb")
        v_sb = persist.tile([P, NBLK, D], f32r, name="v_sb")
        den_sb = persist.tile([P, 8], f32, name="den_sb")

        # ---- fused schedule ----
        # R1: v-eighths with qT / kT-chalf0 chunklets interleaved per kc
        #     (spreads their DMA bursts); psv 4 + pj 2 PSUM banks.
        # R2: attention pairs 0,2 interleaved with kT-chalf1 (whose psum
        #     rides the psT tag); psl 2 + psT 2 + psav 4 = 8 banks.
        # R3: attention pairs 3,1 (DMA long done; pure PE).
        with ExitStack() as phase_ctx:
            pj = phase_ctx.enter_context(
                tc.tile_pool(name="pj", bufs=1, space="PSUM"))
            wvpool = phase_ctx.enter_context(
                tc.tile_pool(name="wvpool", bufs=1))
            xpool = phase_ctx.enter_context(
                tc.tile_pool(name="xpool", bufs=3))
            wv_sb = wvpool.tile([P, KC, D], f32r, name="wv_sb")
            eTs_all = {pr: [] for pr in range(4)}
            daccs_all = {pr: [[], []] for pr in range(4)}
            proj_ps = {}

            def qT_chunklet(qkc):
                if qkc == 0:
                    nc.sync.dma_start(
                        wq_sb[:], wqT.rearrange("(kc p) m -> p kc m", p=P))
                    proj_ps["q"] = [pj.tile([P, 512], f32, tag=f"pj{n}",
                                            name=f"ps_q{n}") for n in range(2)]
                xq = stream.tile([P, NQ], f32r, tag="xq", name=f"xq{qkc}",
                                 bufs=3)
                nc.sync.dma_start(xq[:], xqT[qkc * P:(qkc + 1) * P, :])
                for n in range(2):
                    nc.tensor.matmul(
                        proj_ps["q"][n][:],
                        lhsT=wq_sb[:, qkc, :],
                        rhs=xq[:, n * 512:(n + 1) * 512],
                        start=(qkc == 0), stop=(qkc == KC - 1),
                    )
                if qkc == KC - 1:
                    for n in range(2):
                        nc.any.tensor_copy(
                            qT_sb[:, n * 512:(n + 1) * 512],
                            proj_ps["q"][n][:])

            def kT_chunklet(chalf, kkc, ps_pool=None, ps_tag="pj"):
                if kkc == 0:
                    if chalf == 0:
                        nc.sync.dma_start(
                            wk_sb[:],
                            wkT.rearrange("(kc p) m -> p kc m", p=P))
                    pool_ = ps_pool if ps_pool is not None else pj
                    proj_ps[chalf] = [
                        pool_.tile([P, 512], f32, tag=f"{ps_tag}{n}",
                                   name=f"ps_k{chalf}_{n}") for n in range(2)]
                ps_k = proj_ps[chalf]
                ct = stream.tile([P, NQ], f32r, tag="ct",
                                 name=f"ct{chalf}_{kkc}", bufs=5)
                nc.sync.dma_start(
                    ct[:], cT[kkc * P:(kkc + 1) * P,
                              chalf * NQ:(chalf + 1) * NQ])
                for n in range(2):
                    nc.tensor.matmul(
                        ps_k[n][:],
                        lhsT=wk_sb[:, kkc, :],
                        rhs=ct[:, n * 512:(n + 1) * 512],
                        start=(kkc == 0), stop=(kkc == KC - 1),
                    )
                if kkc == KC - 1:
                    for n in range(2):
                        nc.any.tensor_copy(
                            kT_sb[:, chalf * NQ + n * 512:
                                  chalf * NQ + (n + 1) * 512], ps_k[n][:])

            psv = phase_ctx.enter_context(
                tc.tile_pool(name="psv", bufs=1, space="PSUM"))
            if True:

                xT_r = xT.rearrange("(kc p) s -> p kc s", p=P)

                def v_eighth(se, dma_hook=None, mm_hook=None,
                             defer_copies=False, split_xt=False):
                    # One strided DMA loads all 8 kc chunks of this eighth
                    # (each dma_start costs ~650ns of SP sequencer issue
                    # time, so fewer/bigger DMAs beat per-chunk loads).
                    # split_xt: ve0 loads per-kc so the PE can consume each
                    # (wv[kc], xt[kc]) pair as it lands at startup.
                    if split_xt:
                        xta = xpool.tile([P, KC, 256], f32r, tag="xts",
                                         name=f"xts{se}", bufs=2)
                        for kc in range(KC):
                            if dma_hook is not None:
                                dma_hook(kc)
                            nc.sync.dma_start(
                                xta[:, kc, :],
                                xT[kc * P:(kc + 1) * P,
                                   se * 256:(se + 1) * 256])
                    else:
                        xta = xpool.tile([P, KC, 256], f32r, tag="xt",
                                         name=f"xta{se}")
                        # kc-quarter loads: the kc-ascending matmuls
                        # start as soon as the first quarter lands
                        for h in range(4):
                            nc.sync.dma_start(
                                xta[:, 2 * h:2 * h + 2, :],
                                xT_r[:, 2 * h:2 * h + 2,
                                     se * 256:(se + 1) * 256])
                        if dma_hook is not None:
                            for kc in range(KC):
                                dma_hook(kc)
                    pss = [psv.tile([P, 512], f32, tag=f"psv{i}",
                                    name=f"psv{se}_{i}") for i in range(4)]
                    for kc in range(KC):
                        for n in range(2):
                            for sb in range(2):
                                nc.tensor.matmul(
                                    pss[sb * 2 + n][:],
                                    lhsT=xta[:, kc, sb * P:(sb + 1) * P],
                                    rhs=wv_sb[:, kc, n * 512:(n + 1) * 512],
                                    start=(kc == 0), stop=(kc == KC - 1),
                                )
                        if mm_hook is not None:
                            mm_hook(kc)
                    def flush():
                        for sb in range(2):
                            for n in range(2):
                                nc.any.tensor_copy(
                                    v_sb[:, se * 2 + sb,
                                         n * 512:(n + 1) * 512],
                                    pss[sb * 2 + n][:])
                    if defer_copies:
                        return flush
                    flush()
                    return None

                v_eighth(0, dma_hook=lambda kc: nc.sync.dma_start(
                    wv_sb[:, kc, :], wvT_r[kc]), split_xt=True)
                # qT / kT-chalf0 chunklets spread 2-3 per eighth so each
                # hosting eighth stays DMA-surplus-positive.
                def qh(base):
                    return lambda kc: qT_chunklet(base + kc // 4) \
                        if kc % 4 == 3 else None

                def kh(lst):
                    return lambda kc: kT_chunklet(0, lst[kc // 3]) \
                        if kc % 3 == 1 and kc // 3 < len(lst) else None

                v_eighth(1, mm_hook=qh(0), split_xt=True)
                v_eighth(2, mm_hook=qh(2))
                v_eighth(3, mm_hook=qh(4))
                v_eighth(4, mm_hook=qh(6))
                v_eighth(5, mm_hook=kh([0, 1, 2]))
                v_eighth(6, mm_hook=kh([3, 4, 5]))
                # the SP FIFO tail is xt-slot gated, so these mask loads
                # slip into idle DMA windows without delaying ve7
                nc.sync.dma_start(qmn_sb[:], qmn[:])
                nc.sync.dma_start(dm_sb[:], dmask.rearrange("s p t -> p s t"))
                nc.sync.dma_start(kmb_sb[:], kmb[:])
                v_eighth(7, mm_hook=kh([6, 7]))

        # ---- attention (stage A: logits->exp->transpose; B: AV) ----
        if True:
            apool = ctx.enter_context(tc.tile_pool(name="apool", bufs=5))
            epool = ctx.enter_context(tc.tile_pool(name="epool", bufs=24))
            with tc.tile_pool(name="psl", bufs=2, space="PSUM") as psl_pool, \
                 tc.tile_pool(name="psT", bufs=2, space="PSUM") as psT_pool, \
                 tc.tile_pool(name="psav", bufs=2, space="PSUM") as psav_pool, \
                 tc.tile_pool(name="pjk", bufs=1, space="PSUM") as pjk_pool:

                def stage_a_chunk(pr, j, mid_hook=None):
                    c = PAIR_C[pr]
                    psTs = [psT_pool.tile([P, 256], f32r, tag="psT",
                                          name=f"psT{pr}_{j}_{ks}",
                                          padded_shape=[P, 512])
                            for ks in range(4)]
                    es = []
                    for blk in range(2):
                        slot = pr * 2 + blk
                        psl = psl_pool.tile([P, 512], f32, tag="psl",
                                            name=f"psl{slot}_{j}")
                        nc.tensor.matmul(
                            psl[:],
                            lhsT=qT_sb[:, slot * P:(slot + 1) * P],
                            rhs=kT_sb[:, j * 512:(j + 1) * 512],
                            start=True, stop=True,
                        )
                        sbl = apool.tile([P, 512], f32, tag="sbl",
                                         name=f"sbl{slot}_{j}")
                        add_src = dm_sb[:, slot, :] if j == c - 1 \
                            else kmb_sb[:, j * 512:(j + 1) * 512]
                        nc.vector.tensor_tensor(
                            out=sbl[:], in0=psl[:], in1=add_src, op=ALU.add)
                        e = apool.tile([P, 512], f32r, tag="e",
                                       name=f"e{slot}_{j}")
                        dac = apool.tile([P, 1], f32, tag="dac",
                                         name=f"dac{slot}_{j}", bufs=10)
                        nc.scalar.activation(
                            e[:], sbl[:], AF.Exp,
                            bias=qmn_sb[:, slot:slot + 1], scale=1.0,
                            accum_out=dac[:],
                        )
                        daccs_all[pr][blk].append(dac)
                        es.append(e)
                    # PE work emitted here hides the DVE-add + exp latency
                    # before the transposes need the exp outputs
                    if mid_hook is not None:
                        mid_hook()
                    for blk in range(2):
                        for ks in range(4):
                            nc.tensor.transpose(
                                psTs[ks][:, blk * P:(blk + 1) * P],
                                es[blk][:, ks * P:(ks + 1) * P],
                                ident[:],
                            )
                    for ks in range(4):
                        eT = epool.tile([P, 256], f32r, tag="eT",
                                        name=f"eT{pr}_{j}_{ks}")
                        nc.any.tensor_copy(eT[:], psTs[ks][:])
                        eTs_all[pr].append(eT)

                def stage_b(pr, use_pjk=False, den_dma=False):
                    c = PAIR_C[pr]
                    eTs = eTs_all[pr]
                    for blk in range(2):
                        slot = pr * 2 + blk
                        dl = daccs_all[pr][blk]
                        dst = den_sb[:, slot:slot + 1]
                        if c == 1:
                            nc.any.tensor_copy(dst, dl[0][:])
                        else:
                            nc.vector.tensor_tensor(
                                out=dst, in0=dl[0][:], in1=dl[1][:],
                                op=ALU.add)
                            for d in dl[2:]:
                                nc.vector.tensor_tensor(
                                    out=dst, in0=dst, in1=d[:], op=ALU.add)
                    if den_dma:
                        # all 8 den slots are final here; flush during the AV
                        nc.sync.dma_start(den[:], den_sb[:])
                    for dmc in range(8):
                        if use_pjk and dmc % 4 >= 2:
                            # kT-chalf1's pjk banks are dead by R3: reuse
                            # them as two extra AV slots (depth 4 pipeline)
                            psav = pjk_pool.tile(
                                [P, 512], f32, tag=f"pjk{dmc % 2}",
                                name=f"psav{pr}_{dmc}")[:, :256]
                        else:
                            psav = psav_pool.tile([P, 256], f32, tag="psav",
                                                  name=f"psav{pr}_{dmc}")
                        for kb in range(4 * c):
                            nc.tensor.matmul(
                                psav[:],
                                lhsT=v_sb[:, kb, dmc * P:(dmc + 1) * P],
                                rhs=eTs[kb][:],
                                start=(kb == 0), stop=(kb == 4 * c - 1),
                            )
                        osb = apool.tile([P, 256], f32, tag="osb",
                                         name=f"osb{pr}_{dmc}")
                        if den_dma and dmc == 7:
                            # last output copy of the kernel: pin to DVE so
                            # it runs beside ACT's previous copy at the tail
                            nc.vector.tensor_copy(osb[:], psav[:])
                        else:
                            nc.any.tensor_copy(osb[:], psav[:])
                        nc.sync.dma_start(
                            outT[dmc * P:(dmc + 1) * P,
                                 pr * 256:(pr + 1) * 256], osb[:])

                # R2: pairs 0 and 2 interleaved with kT chalf1 (psum on psT tag)
                def kt1_first():
                    for kkc in range(4):
                        kT_chunklet(1, kkc, ps_pool=pjk_pool, ps_tag="pjk")

                def kt1_second():
                    for kkc in range(4, 8):
                        kT_chunklet(1, kkc, ps_pool=pjk_pool, ps_tag="pjk")

                stage_a_chunk(0, 0, mid_hook=kt1_first)
                stage_a_chunk(2, 0)
                stage_b(0)
                stage_a_chunk(2, 1, mid_hook=kt1_second)
                stage_b(2)
                # R3: pair 1 then 3, A3 interleaved before B1
                for j in range(4):
                    stage_a_chunk(1, j)
                stage_a_chunk(3, 0)
                stage_b(1, use_pjk=True)
                stage_a_chunk(3, 1)
                stage_a_chunk(3, 2)
                stage_b(3, use_pjk=True, den_dma=True)


    nc.compile()
    return nc


def _get_nc():
    with _BUILD_LOCK:
        if "nc" not in _CACHE:
            _CACHE["nc"] = _build()
        return _CACHE["nc"]


def kernel(x, cross, Wq, Wk, Wv, mask):
    from concourse import bass_utils

    nc = _get_nc()

    x = np.asarray(x, dtype=np.float32)
    cross = np.asarray(cross, dtype=np.float32)
    scale = 1.0 / math.sqrt(DA)
    wqT_h = np.ascontiguousarray((np.asarray(Wq, np.float32) * scale).T)
    wkT_h = np.ascontiguousarray(np.asarray(Wk, np.float32).T)
    wvT_h = np.ascontiguousarray(np.asarray(Wv, np.float32).T)
    mf = np.asarray(mask).astype(np.float32)  # [B, S]

    karange = np.arange(S)
    in_maps = []
    rows_per_core = []
    for core in range(NCORES):
        b, p = divmod(core, 2)
        blocks = STRIPS[p]
        rows = np.concatenate([np.arange(g * P, (g + 1) * P) for g in blocks])
        rows_per_core.append((b, rows))
        mb = mf[b]
        kneg = (-BIG * (1.0 - mb)).astype(np.float32)  # [S]
        kmb_h = np.ascontiguousarray(
            np.broadcast_to(kneg[:1536], (P, 1536))).astype(ml_dtypes.bfloat16)
        mq = mb[rows]  # [1024]
        qmn_h = np.ascontiguousarray(
            (-BIG * (1.0 - mq)).reshape(8, P).T)  # [128, 8]
        dm_h = np.empty((8, P, 512), np.float32)
        for s, g in enumerate(blocks):
            c = PAIR_C[s // 2]
            k0 = (c - 1) * 512
            kk = karange[k0:k0 + 512]
            qq = g * P + np.arange(P)
            mqs = mq[s * P:(s + 1) * P]
            t = np.broadcast_to(kneg[k0:k0 + 512], (P, 512)).copy()
            t += -BIG * (kk[None, :] > qq[:, None])
            t += (2.0 * BIG * (1.0 - mqs))[:, None] * (kk[None, :] == qq[:, None])
            dm_h[s] = t
        in_maps.append({
            "xT": np.ascontiguousarray(x[b].T),
            "cT": np.ascontiguousarray(cross[b].T),
            "xqT": np.ascontiguousarray(x[b][rows].T),
            "wqT": wqT_h,
            "wkT": wkT_h,
            "wvT": wvT_h,
            "kmb": kmb_h,
            "qmn": qmn_h,
            "dmask": dm_h.astype(ml_dtypes.bfloat16),
        })

    _CACHE["in_maps"] = in_maps
    res = bass_utils.run_bass_kernel_spmd(
        nc, in_maps, core_ids=list(range(NCORES)))

    out = np.empty((B, S, D), np.float32)
    for core in range(NCORES):
        b, rows = rows_per_core[core]
        r = res.results[core]
        o = r["outT"].T  # [1024 q, 1024 dm]
        denf = r["den"].T.reshape(-1)  # [1024] strip-ordered
        out[b, rows] = o / denf[:, None]
    return out



# revision 4
# speedup vs baseline: 1.1893x; 1.1893x over previous
"""Trainium2 Bass kernel for nn_Attention_42288247996512 (sparse causal cross-attention).

reference:
  q = x @ Wq.T; k = cross @ Wk.T; v = x @ Wv.T
  logits = q @ k.T  (causal mask; padding mask m_q*m_k + eye > 0)
  out = softmax(logits / sqrt(128)) @ v

Sharding: 8 cores = 4 batches x 2 query-strips (SPMD). Each strip is 8 query
blocks (128 rows) grouped into 4 pairs with identical causal-chunk structure
on both strips.

Key algebraic optimization vs the v-projection formulation: reassociate
  attn @ (x @ Wv.T)  ==  (attn @ x) @ Wv.T
Each core has NQ=1024 query rows but would need all S=2048 key rows of v, so
projecting t = attn@x (1024 rows) instead of v (2048 rows) halves the
projection matmul work per core (131072 -> 65536 PE cycles).

All streamed operands are bf16 (halves DMA bytes; exp/transpose run at
1 cycle/row instead of 1.5 for f32r transposes). PSUM accumulation stays f32.
Host does layout packs/bf16 casts, additive-mask building, and the final
denominator divide + scatter (as in the baseline kernel).
"""
import math
import threading

import ml_dtypes
import numpy as np

B, S, D, DA = 4, 2048, 1024, 128
P = 128
NCORES = 8
BIG = 32768.0  # power of two: exactly representable in bf16
NBLK = S // P  # 16 key blocks per batch
NQ = 1024      # query rows per core strip
KC = D // P    # 8 contraction chunks of 128

# strips: pairs of adjacent blocks, same chunk-count multiset on both strips
STRIPS = [
    [0, 1, 14, 15, 6, 7, 8, 9],
    [2, 3, 12, 13, 4, 5, 10, 11],
]
PAIR_C = [1, 4, 2, 3]  # 512-wide key chunks per pair (same for both strips)

_BUILD_LOCK = threading.Lock()
_CACHE: dict = {}


def _build():
    from contextlib import ExitStack

    import concourse.bass as bass
    import concourse.mybir as mybir
    import concourse.tile as tile
    from concourse import bacc
    from concourse.masks import make_identity

    dt = mybir.dt
    f32 = dt.float32
    bf16 = dt.bfloat16
    AF = mybir.ActivationFunctionType
    ALU = mybir.AluOpType

    nc = bacc.Bacc("TRN2", target_bir_lowering=False, debug=False)

    # DRAM inputs. Weight tensors are host-packed to [P, ...] so every DMA row
    # is a contiguous >=512B run (full DMA rate).
    xk = nc.dram_tensor("xk", [S, D], bf16, kind="ExternalInput").ap()
    cT = nc.dram_tensor("cT", [D, S], bf16, kind="ExternalInput").ap()
    xqT = nc.dram_tensor("xqT", [D, NQ], bf16, kind="ExternalInput").ap()
    wqp = nc.dram_tensor("wqp", [P, KC, DA], bf16, kind="ExternalInput").ap()
    wkp = nc.dram_tensor("wkp", [P, KC, DA], bf16, kind="ExternalInput").ap()
    wvp = nc.dram_tensor("wvp", [P, KC, D], bf16, kind="ExternalInput").ap()
    # additive masks in bf16 (values are sums of +-2^15/2^16: exact in bf16)
    kmb = nc.dram_tensor("kmb", [P, 1536], bf16, kind="ExternalInput").ap()
    qmn = nc.dram_tensor("qmn", [P, 8], f32, kind="ExternalInput").ap()
    dmask = nc.dram_tensor("dmask", [8, P, 512], bf16, kind="ExternalInput").ap()

    outT = nc.dram_tensor("outT", [D, NQ], f32, kind="ExternalOutput").ap()
    den = nc.dram_tensor("den", [P, 8], f32, kind="ExternalOutput").ap()

    xk_r = xk.rearrange("(kb p) d -> p kb d", p=P)
    cT_r = cT.rearrange("(kc p) s -> p kc s", p=P)
    xqT_r = xqT.rearrange("(kc p) q -> p kc q", p=P)
    outT_r = outT.rearrange("(dmc p) q -> p dmc q", p=P)

    with tile.TileContext(nc) as tc, ExitStack() as ctx:
        const = ctx.enter_context(tc.tile_pool(name="const", bufs=1))
        persist = ctx.enter_context(tc.tile_pool(name="persist", bufs=1))
        stream = ctx.enter_context(tc.tile_pool(name="stream", bufs=2))
        apool = ctx.enter_context(tc.tile_pool(name="apool", bufs=4))
        epool = ctx.enter_context(tc.tile_pool(name="epool", bufs=24))

        ident_f32 = const.tile([P, P], f32, name="ident_f32")
        make_identity(nc, ident_f32)
        ident = const.tile([P, P], bf16, name="ident")
        nc.vector.tensor_copy(ident[:], ident_f32[:])

        wq_sb = const.tile([P, KC, DA], bf16, name="wq_sb")
        wk_sb = const.tile([P, KC, DA], bf16, name="wk_sb")
        wv_sb = const.tile([P, KC, D], bf16, name="wv_sb")
        kmb_sb = const.tile([P, 1536], bf16, name="kmb_sb")
        qmn_sb = const.tile([P, 8], f32, name="qmn_sb")
        dm_sb = const.tile([P, 8, 512], bf16, name="dm_sb")

        kT_sb = persist.tile([P, S], bf16, name="kT_sb")
        qT_sb = persist.tile([P, NQ], bf16, name="qT_sb")
        xk_sb = persist.tile([P, NBLK, D], bf16, name="xk_sb")
        tT_sb = persist.tile([P, KC, NQ], bf16, name="tT_sb")
        den_sb = persist.tile([P, 8], f32, name="den_sb")

        eTs_all = {pr: [] for pr in range(4)}
        daccs_all = {pr: [[], []] for pr in range(4)}

        # PSUM: 4 pools x 2 bufs x 2KB = all 8 banks.
        psl_pool = ctx.enter_context(
            tc.tile_pool(name="psl", bufs=2, space="PSUM"))
        psT_pool = ctx.enter_context(
            tc.tile_pool(name="psT", bufs=2, space="PSUM"))
        psax_pool = ctx.enter_context(
            tc.tile_pool(name="psax", bufs=2, space="PSUM"))
        psw_pool = ctx.enter_context(
            tc.tile_pool(name="psw", bufs=2, space="PSUM"))

        # ---- projections (DMA emission order == SP FIFO delivery order) ----
        # qT = (Wq/sqrt(128)) @ x_strip.T, accumulated over 8 kc chunks.
        nc.sync.dma_start(wq_sb[:], wqp)
        ps_q = [psl_pool.tile([P, 512], f32, tag="psl", name=f"ps_q{n}")
                for n in range(2)]
        for kc in range(KC):
            xqt = stream.tile([P, NQ], bf16, tag="xq", name=f"xq{kc}", bufs=3)
            nc.sync.dma_start(xqt[:], xqT_r[:, kc, :])
            for n in range(2):
                nc.tensor.matmul(
                    ps_q[n][:],
                    lhsT=wq_sb[:, kc, :],
                    rhs=xqt[:, n * 512:(n + 1) * 512],
                    start=(kc == 0), stop=(kc == KC - 1),
                )
        for n in range(2):
            nc.any.tensor_copy(qT_sb[:, n * 512:(n + 1) * 512], ps_q[n][:])
        nc.sync.dma_start(wk_sb[:], wkp)

        # kT = Wk @ cross.T, computed per 512-key chunk j so attention on
        # early chunks can start while later cT chunks stream in.
        def kT_chunk(j):
            ctj = stream.tile([P, KC, 512], bf16, tag="ct", name=f"ct{j}",
                              bufs=2)
            nc.sync.dma_start(ctj[:], cT_r[:, :, j * 512:(j + 1) * 512])
            ps_k = psax_pool.tile([P, 512], f32, tag="psax", name=f"ps_k{j}")
            for kc in range(KC):
                nc.tensor.matmul(
                    ps_k[:],
                    lhsT=wk_sb[:, kc, :],
                    rhs=ctj[:, kc, :],
                    start=(kc == 0), stop=(kc == KC - 1),
                )
            nc.any.tensor_copy(kT_sb[:, j * 512:(j + 1) * 512], ps_k[:])

        kT_chunk(0)
        nc.sync.dma_start(qmn_sb[:], qmn[:])
        nc.sync.dma_start(dm_sb[:], dmask.rearrange("s p t -> p s t"))
        nc.sync.dma_start(kmb_sb[:], kmb[:])
        nc.sync.dma_start(xk_sb[:, 0:4, :], xk_r[:, 0:4, :])

        # ---- attention stages ----
        def stage_a_chunk(pr, j, mid_hook=None):
            c = PAIR_C[pr]
            psTs = [psT_pool.tile([P, 256], bf16, tag="psT",
                                  name=f"psT{pr}_{j}_{ks}",
                                  padded_shape=[P, 1024])
                    for ks in range(4)]
            es = []
            for blk in range(2):
                slot = pr * 2 + blk
                psl = psl_pool.tile([P, 512], f32, tag="psl",
                                    name=f"psl{slot}_{j}")
                nc.tensor.matmul(
                    psl[:],
                    lhsT=qT_sb[:, slot * P:(slot + 1) * P],
                    rhs=kT_sb[:, j * 512:(j + 1) * 512],
                    start=True, stop=True,
                )
                sbl = apool.tile([P, 512], f32, tag="sbl",
                                 name=f"sbl{slot}_{j}")
                add_src = dm_sb[:, slot, :] if j == c - 1 \
                    else kmb_sb[:, j * 512:(j + 1) * 512]
                nc.vector.tensor_tensor(
                    out=sbl[:], in0=psl[:], in1=add_src, op=ALU.add)
                e = apool.tile([P, 512], bf16, tag="e", name=f"e{slot}_{j}")
                dac = apool.tile([P, 1], f32, tag="dac",
                                 name=f"dac{slot}_{j}", bufs=12)
                nc.scalar.activation(
                    e[:], sbl[:], AF.Exp,
                    bias=qmn_sb[:, slot:slot + 1], scale=1.0,
                    accum_out=dac[:],
                )
                daccs_all[pr][blk].append(dac)
                es.append(e)
            # PE work emitted here hides the DVE-add + exp latency
            if mid_hook is not None:
                mid_hook()
            for blk in range(2):
                for ks in range(4):
                    nc.tensor.transpose(
                        psTs[ks][:, blk * P:(blk + 1) * P],
                        es[blk][:, ks * P:(ks + 1) * P],
                        ident[:],
                    )
            for ks in range(4):
                eT = epool.tile([P, 256], bf16, tag="eT",
                                name=f"eT{pr}_{j}_{ks}")
                nc.any.tensor_copy(eT[:], psTs[ks][:])
                eTs_all[pr].append(eT)

        def stage_den(pr):
            for blk in range(2):
                slot = pr * 2 + blk
                dl = daccs_all[pr][blk]
                dst = den_sb[:, slot:slot + 1]
                if len(dl) == 1:
                    nc.any.tensor_copy(dst, dl[0][:])
                else:
                    nc.vector.tensor_tensor(
                        out=dst, in0=dl[0][:], in1=dl[1][:], op=ALU.add)
                    for d in dl[2:]:
                        nc.vector.tensor_tensor(
                            out=dst, in0=dst, in1=d[:], op=ALU.add)

        def stage_ax(pr, mid_hook=None):
            # t.T[xd, q] += x[kb].T-contraction with eT: per xd chunk,
            # accumulate over the pair's key blocks.
            c = PAIR_C[pr]
            stage_den(pr)
            eTs = eTs_all[pr]
            for xdc in range(KC):
                psax = psax_pool.tile([P, 256], f32, tag="psax",
                                      name=f"psax{pr}_{xdc}",
                                      padded_shape=[P, 512])
                for kb in range(4 * c):
                    nc.tensor.matmul(
                        psax[:],
                        lhsT=xk_sb[:, kb, xdc * P:(xdc + 1) * P],
                        rhs=eTs[kb][:],
                        start=(kb == 0), stop=(kb == 4 * c - 1),
                    )
                nc.any.tensor_copy(
                    tT_sb[:, xdc, pr * 256:(pr + 1) * 256], psax[:])
                if mid_hook is not None and xdc == 3:
                    mid_hook()

        def stage_twv(pr, dm_half=None, den_dma=False):
            # out.T[dm, q] = Wv.T.T @ t.T for this pair's 256 query columns
            halves = [0, 1] if dm_half is None else [dm_half]
            osb = _osbs.setdefault(
                pr, apool.tile([P, KC, 256], f32, tag="osb",
                               name=f"osb{pr}", bufs=2))
            if den_dma:
                nc.sync.dma_start(den[:], den_sb[:])
            for h in halves:
                for dmc in range(4 * h, 4 * h + 4):
                    psw = psw_pool.tile([P, 256], f32, tag="psw",
                                        name=f"psw{pr}_{dmc}",
                                        padded_shape=[P, 512])
                    for xdc in range(KC):
                        nc.tensor.matmul(
                            psw[:],
                            lhsT=wv_sb[:, xdc, dmc * P:(dmc + 1) * P],
                            rhs=tT_sb[:, xdc, pr * 256:(pr + 1) * 256],
                            start=(xdc == 0), stop=(xdc == KC - 1),
                        )
                    nc.any.tensor_copy(osb[:, dmc, :], psw[:])
            if halves[-1] == 1:
                nc.sync.dma_start(
                    outT_r[:, :, pr * 256:(pr + 1) * 256], osb[:])

        _osbs = {}

        # ---- fused schedule (PE emission order tuned to DMA arrivals) ----
        stage_a_chunk(0, 0, mid_hook=lambda: kT_chunk(1))
        stage_a_chunk(2, 0)
        stage_ax(0)
        nc.sync.dma_start(xk_sb[:, 4:8, :], xk_r[:, 4:8, :])
        stage_a_chunk(2, 1, mid_hook=lambda: kT_chunk(2))
        stage_ax(2)
        nc.sync.dma_start(wv_sb[:, :, 0:512], wvp[:, :, 0:512])
        stage_a_chunk(1, 0)
        stage_a_chunk(1, 1)
        stage_twv(0, dm_half=0)
        stage_a_chunk(1, 2, mid_hook=lambda: kT_chunk(3))
        nc.sync.dma_start(wv_sb[:, :, 512:1024], wvp[:, :, 512:1024])
        stage_twv(0, dm_half=1)
        stage_a_chunk(1, 3)
        nc.sync.dma_start(xk_sb[:, 8:12, :], xk_r[:, 8:12, :])
        nc.sync.dma_start(xk_sb[:, 12:16, :], xk_r[:, 12:16, :])
        stage_twv(2, dm_half=0)
        stage_twv(2, dm_half=1)
        stage_ax(1, mid_hook=lambda: stage_a_chunk(3, 0))
        stage_twv(1, dm_half=0)
        stage_a_chunk(3, 1)
        stage_twv(1, dm_half=1)
        stage_a_chunk(3, 2)
        stage_ax(3)
        stage_twv(3, den_dma=True)

    nc.compile()
    return nc


def _get_nc():
    with _BUILD_LOCK:
        if "nc" not in _CACHE:
            _CACHE["nc"] = _build()
        return _CACHE["nc"]


def kernel(x, cross, Wq, Wk, Wv, mask):
    from concourse import bass_utils

    nc = _get_nc()

    bf = ml_dtypes.bfloat16
    x = np.asarray(x, dtype=np.float32)
    cross = np.asarray(cross, dtype=np.float32)
    scale = 1.0 / math.sqrt(DA)

    def pack_w(wT, m_cols):
        # [D, m] -> [P, KC, m] with [p, kc, m] = wT[kc*128 + p, m]
        return np.ascontiguousarray(
            wT.reshape(KC, P, m_cols).transpose(1, 0, 2)).astype(bf)

    wqp_h = pack_w((np.asarray(Wq, np.float32) * scale).T, DA)
    wkp_h = pack_w(np.asarray(Wk, np.float32).T, DA)
    wvp_h = pack_w(np.asarray(Wv, np.float32).T, D)
    mf = np.asarray(mask).astype(np.float32)  # [B, S]

    karange = np.arange(S)
    in_maps = []
    rows_per_core = []
    for core in range(NCORES):
        b, p = divmod(core, 2)
        blocks = STRIPS[p]
        rows = np.concatenate([np.arange(g * P, (g + 1) * P) for g in blocks])
        rows_per_core.append((b, rows))
        mb = mf[b]
        kneg = (-BIG * (1.0 - mb)).astype(np.float32)  # [S]
        kmb_h = np.ascontiguousarray(
            np.broadcast_to(kneg[:1536], (P, 1536))).astype(bf)
        mq = mb[rows]  # [1024]
        qmn_h = np.ascontiguousarray(
            (-BIG * (1.0 - mq)).reshape(8, P).T)  # [128, 8]
        dm_h = np.empty((8, P, 512), np.float32)
        for s, g in enumerate(blocks):
            c = PAIR_C[s // 2]
            k0 = (c - 1) * 512
            kk = karange[k0:k0 + 512]
            qq = g * P + np.arange(P)
            mqs = mq[s * P:(s + 1) * P]
            t = np.broadcast_to(kneg[k0:k0 + 512], (P, 512)).copy()
            t += -BIG * (kk[None, :] > qq[:, None])
            t += (2.0 * BIG * (1.0 - mqs))[:, None] * (kk[None, :] == qq[:, None])
            dm_h[s] = t
        in_maps.append({
            "xk": np.ascontiguousarray(x[b]).astype(bf),
            "cT": np.ascontiguousarray(cross[b].T).astype(bf),
            "xqT": np.ascontiguousarray(x[b][rows].T).astype(bf),
            "wqp": wqp_h,
            "wkp": wkp_h,
            "wvp": wvp_h,
            "kmb": kmb_h,
            "qmn": qmn_h,
            "dmask": dm_h.astype(bf),
        })

    _CACHE["in_maps"] = in_maps
    res = bass_utils.run_bass_kernel_spmd(
        nc, in_maps, core_ids=list(range(NCORES)))

    out = np.empty((B, S, D), np.float32)
    for core in range(NCORES):
        b, rows = rows_per_core[core]
        r = res.results[core]
        o = r["outT"].T  # [1024 q, 1024 dm]
        denf = r["den"].T.reshape(-1)  # [1024] strip-ordered
        out[b, rows] = o / denf[:, None]
    return out


# revision 8
# speedup vs baseline: 1.2044x; 1.0127x over previous
"""Trainium2 Bass kernel for nn_Attention_42288247996512 (sparse causal cross-attention).

reference:
  q = x @ Wq.T; k = cross @ Wk.T; v = x @ Wv.T
  logits = q @ k.T  (causal mask; padding mask m_q*m_k + eye > 0)
  out = softmax(logits / sqrt(128)) @ v

Sharding: 8 cores = 4 batches x 2 query-strips (SPMD). Each strip is 8 query
blocks (128 rows) grouped into 4 pairs with identical causal-chunk structure
on both strips.

Key algebraic optimization vs the v-projection formulation: reassociate
  attn @ (x @ Wv.T)  ==  (attn @ x) @ Wv.T
Each core has NQ=1024 query rows but would need all S=2048 key rows of v, so
projecting t = attn@x (1024 rows) instead of v (2048 rows) halves the
projection matmul work per core (131072 -> 65536 PE cycles).

All streamed operands are bf16 (halves DMA bytes; exp/transpose run at
1 cycle/row instead of 1.5 for f32r transposes). PSUM accumulation stays f32.
Host does layout packs/bf16 casts, additive-mask building, and the final
denominator divide + scatter (as in the baseline kernel).
"""
import math
import threading

import ml_dtypes
import numpy as np

B, S, D, DA = 4, 2048, 1024, 128
P = 128
NCORES = 8
BIG = 32768.0  # power of two: exactly representable in bf16
NBLK = S // P  # 16 key blocks per batch
NQ = 1024      # query rows per core strip
KC = D // P    # 8 contraction chunks of 128

# strips: pairs of adjacent blocks, same chunk-count multiset on both strips
STRIPS = [
    [0, 1, 14, 15, 6, 7, 8, 9],
    [2, 3, 12, 13, 4, 5, 10, 11],
]
PAIR_C = [1, 4, 2, 3]  # 512-wide key chunks per pair (same for both strips)

_BUILD_LOCK = threading.Lock()
_CACHE: dict = {}


def _build():
    from contextlib import ExitStack

    import concourse.bass as bass
    import concourse.mybir as mybir
    import concourse.tile as tile
    from concourse import bacc
    from concourse.masks import make_identity

    dt = mybir.dt
    f32 = dt.float32
    bf16 = dt.bfloat16
    AF = mybir.ActivationFunctionType
    ALU = mybir.AluOpType

    nc = bacc.Bacc("TRN2", target_bir_lowering=False, debug=False)

    # DRAM inputs. Weight tensors are host-packed to [P, ...] so every DMA row
    # is a contiguous >=512B run (full DMA rate).
    xk = nc.dram_tensor("xk", [S, D], bf16, kind="ExternalInput").ap()
    cT = nc.dram_tensor("cT", [D, S], bf16, kind="ExternalInput").ap()
    xqT = nc.dram_tensor("xqT", [D, NQ], bf16, kind="ExternalInput").ap()
    wqp = nc.dram_tensor("wqp", [P, KC, DA], bf16, kind="ExternalInput").ap()
    wkp = nc.dram_tensor("wkp", [P, KC, DA], bf16, kind="ExternalInput").ap()
    wvp = nc.dram_tensor("wvp", [P, KC, D], bf16, kind="ExternalInput").ap()
    # additive masks in bf16 (values are sums of +-2^15/2^16: exact in bf16)
    kmb = nc.dram_tensor("kmb", [P, 1536], bf16, kind="ExternalInput").ap()
    qmn = nc.dram_tensor("qmn", [P, 8], f32, kind="ExternalInput").ap()
    dmask = nc.dram_tensor("dmask", [8, P, 512], bf16, kind="ExternalInput").ap()

    outT = nc.dram_tensor("outT", [D, NQ], f32, kind="ExternalOutput").ap()
    den = nc.dram_tensor("den", [P, 8], f32, kind="ExternalOutput").ap()

    xk_r = xk.rearrange("(kb p) d -> p kb d", p=P)
    cT_r = cT.rearrange("(kc p) s -> p kc s", p=P)
    xqT_r = xqT.rearrange("(kc p) q -> p kc q", p=P)
    outT_r = outT.rearrange("(dmc p) q -> p dmc q", p=P)

    with tile.TileContext(nc) as tc, ExitStack() as ctx:
        const = ctx.enter_context(tc.tile_pool(name="const", bufs=1))
        persist = ctx.enter_context(tc.tile_pool(name="persist", bufs=1))
        stream = ctx.enter_context(tc.tile_pool(name="stream", bufs=2))
        apool = ctx.enter_context(tc.tile_pool(name="apool", bufs=4))
        epool = ctx.enter_context(tc.tile_pool(name="epool", bufs=24))

        ident_f32 = const.tile([P, P], f32, name="ident_f32")
        make_identity(nc, ident_f32)
        ident = const.tile([P, P], bf16, name="ident")
        nc.vector.tensor_copy(ident[:], ident_f32[:])

        wq_sb = const.tile([P, KC, DA], bf16, name="wq_sb")
        wk_sb = const.tile([P, KC, DA], bf16, name="wk_sb")
        wv_sb = const.tile([P, KC, D], bf16, name="wv_sb")
        kmb_sb = const.tile([P, 1536], bf16, name="kmb_sb")
        qmn_sb = const.tile([P, 8], f32, name="qmn_sb")
        dm_sb = const.tile([P, 8, 512], bf16, name="dm_sb")

        kT_sb = persist.tile([P, S], bf16, name="kT_sb")
        qT_sb = persist.tile([P, NQ], bf16, name="qT_sb")
        xk_sb = persist.tile([P, NBLK, D], bf16, name="xk_sb")
        tT_sb = persist.tile([P, KC, NQ], bf16, name="tT_sb")
        den_sb = persist.tile([P, 8], f32, name="den_sb")

        eTs_all = {pr: [] for pr in range(4)}
        daccs_all = {pr: [[], []] for pr in range(4)}

        # PSUM: 4 pools x 2 bufs x 2KB = all 8 banks.
        psl_pool = ctx.enter_context(
            tc.tile_pool(name="psl", bufs=2, space="PSUM"))
        psT_pool = ctx.enter_context(
            tc.tile_pool(name="psT", bufs=2, space="PSUM"))
        psax_pool = ctx.enter_context(
            tc.tile_pool(name="psax", bufs=2, space="PSUM"))
        psw_pool = ctx.enter_context(
            tc.tile_pool(name="psw", bufs=2, space="PSUM"))

        # ---- projections (DMA emission order == SP FIFO delivery order) ----
        # kT = Wk @ cross.T, computed per 512-key chunk j so attention on
        # early chunks can start while later cT chunks stream in.
        def kT_chunk(j):
            ctj = stream.tile([P, KC, 512], bf16, tag="ct", name=f"ct{j}",
                              bufs=2)
            nc.sync.dma_start(ctj[:], cT_r[:, :, j * 512:(j + 1) * 512])
            ps_k = psax_pool.tile([P, 512], f32, tag="psax", name=f"ps_k{j}")
            for kc in range(KC):
                nc.tensor.matmul(
                    ps_k[:],
                    lhsT=wk_sb[:, kc, :],
                    rhs=ctj[:, kc, :],
                    start=(kc == 0), stop=(kc == KC - 1),
                )
            nc.any.tensor_copy(kT_sb[:, j * 512:(j + 1) * 512], ps_k[:])

        # qT = (Wq/sqrt(128)) @ x_strip.T, accumulated over 8 kc chunks;
        # kT chunk 0 is slipped in after xq-kc0 so the PE has dense work
        # while the xq tail streams in.
        nc.sync.dma_start(wq_sb[:], wqp)
        ps_q = [psl_pool.tile([P, 512], f32, tag="psl", name=f"ps_q{n}")
                for n in range(2)]

        def qT_chunk(kc):
            xqt = stream.tile([P, NQ], bf16, tag="xq", name=f"xq{kc}", bufs=3)
            nc.sync.dma_start(xqt[:], xqT_r[:, kc, :])
            for n in range(2):
                nc.tensor.matmul(
                    ps_q[n][:],
                    lhsT=wq_sb[:, kc, :],
                    rhs=xqt[:, n * 512:(n + 1) * 512],
                    start=(kc == 0), stop=(kc == KC - 1),
                )

        qT_chunk(0)
        nc.sync.dma_start(wk_sb[:], wkp)
        kT_chunk(0)
        for kc in range(1, KC):
            qT_chunk(kc)
        for n in range(2):
            nc.any.tensor_copy(qT_sb[:, n * 512:(n + 1) * 512], ps_q[n][:])
        nc.sync.dma_start(qmn_sb[:], qmn[:])
        nc.sync.dma_start(dm_sb[:], dmask.rearrange("s p t -> p s t"))
        nc.sync.dma_start(kmb_sb[:], kmb[:])
        nc.sync.dma_start(xk_sb[:, 0:4, :], xk_r[:, 0:4, :])

        # ---- attention stages ----
        def stage_a_chunk(pr, j, mid_hook=None):
            c = PAIR_C[pr]
            psTs = [psT_pool.tile([P, 256], bf16, tag="psT",
                                  name=f"psT{pr}_{j}_{ks}",
                                  padded_shape=[P, 1024])
                    for ks in range(4)]
            es = []
            for blk in range(2):
                slot = pr * 2 + blk
                psl = psl_pool.tile([P, 512], f32, tag="psl",
                                    name=f"psl{slot}_{j}")
                nc.tensor.matmul(
                    psl[:],
                    lhsT=qT_sb[:, slot * P:(slot + 1) * P],
                    rhs=kT_sb[:, j * 512:(j + 1) * 512],
                    start=True, stop=True,
                )
                sbl = apool.tile([P, 512], f32, tag="sbl",
                                 name=f"sbl{slot}_{j}")
                add_src = dm_sb[:, slot, :] if j == c - 1 \
                    else kmb_sb[:, j * 512:(j + 1) * 512]
                nc.vector.tensor_tensor(
                    out=sbl[:], in0=psl[:], in1=add_src, op=ALU.add)
                e = apool.tile([P, 512], bf16, tag="e", name=f"e{slot}_{j}")
                dac = apool.tile([P, 1], f32, tag="dac",
                                 name=f"dac{slot}_{j}", bufs=12)
                nc.scalar.activation(
                    e[:], sbl[:], AF.Exp,
                    bias=qmn_sb[:, slot:slot + 1], scale=1.0,
                    accum_out=dac[:],
                )
                daccs_all[pr][blk].append(dac)
                es.append(e)
            # PE work emitted here hides the DVE-add + exp latency
            if mid_hook is not None:
                mid_hook()
            for blk in range(2):
                for ks in range(4):
                    nc.tensor.transpose(
                        psTs[ks][:, blk * P:(blk + 1) * P],
                        es[blk][:, ks * P:(ks + 1) * P],
                        ident[:],
                    )
            for ks in range(4):
                eT = epool.tile([P, 256], bf16, tag="eT",
                                name=f"eT{pr}_{j}_{ks}")
                nc.any.tensor_copy(eT[:], psTs[ks][:])
                eTs_all[pr].append(eT)

        def stage_den(pr):
            for blk in range(2):
                slot = pr * 2 + blk
                dl = daccs_all[pr][blk]
                dst = den_sb[:, slot:slot + 1]
                if len(dl) == 1:
                    nc.any.tensor_copy(dst, dl[0][:])
                else:
                    nc.vector.tensor_tensor(
                        out=dst, in0=dl[0][:], in1=dl[1][:], op=ALU.add)
                    for d in dl[2:]:
                        nc.vector.tensor_tensor(
                            out=dst, in0=dst, in1=d[:], op=ALU.add)

        def stage_ax(pr, mid_hook=None):
            # t.T[xd, q] += x[kb].T-contraction with eT: per xd chunk,
            # accumulate over the pair's key blocks.
            c = PAIR_C[pr]
            stage_den(pr)
            eTs = eTs_all[pr]
            for xdc in range(KC):
                psax = psax_pool.tile([P, 256], f32, tag="psax",
                                      name=f"psax{pr}_{xdc}",
                                      padded_shape=[P, 512])
                for kb in range(4 * c):
                    nc.tensor.matmul(
                        psax[:],
                        lhsT=xk_sb[:, kb, xdc * P:(xdc + 1) * P],
                        rhs=eTs[kb][:],
                        start=(kb == 0), stop=(kb == 4 * c - 1),
                    )
                nc.any.tensor_copy(
                    tT_sb[:, xdc, pr * 256:(pr + 1) * 256], psax[:])
                if mid_hook is not None and xdc == 3:
                    mid_hook()

        def stage_twv(pr, dm_half=None, den_dma=False):
            # out.T[dm, q] = Wv.T.T @ t.T for this pair's 256 query columns
            halves = [0, 1] if dm_half is None else [dm_half]
            osb = _osbs.setdefault(
                pr, apool.tile([P, KC, 256], f32, tag="osb",
                               name=f"osb{pr}", bufs=2))
            if den_dma:
                nc.sync.dma_start(den[:], den_sb[:])
            for h in halves:
                for dmc in range(4 * h, 4 * h + 4):
                    psw = psw_pool.tile([P, 256], f32, tag="psw",
                                        name=f"psw{pr}_{dmc}",
                                        padded_shape=[P, 512])
                    for xdc in range(KC):
                        nc.tensor.matmul(
                            psw[:],
                            lhsT=wv_sb[:, xdc, dmc * P:(dmc + 1) * P],
                            rhs=tT_sb[:, xdc, pr * 256:(pr + 1) * 256],
                            start=(xdc == 0), stop=(xdc == KC - 1),
                        )
                    # alternate engines so the final copies drain in parallel
                    if dmc % 2 == 0:
                        nc.vector.tensor_copy(osb[:, dmc, :], psw[:])
                    else:
                        nc.scalar.copy(osb[:, dmc, :], psw[:])
                nc.sync.dma_start(
                    outT_r[:, 4 * h:4 * h + 4, pr * 256:(pr + 1) * 256],
                    osb[:, 4 * h:4 * h + 4, :])

        _osbs = {}

        # ---- fused schedule (PE emission order tuned to DMA arrivals) ----
        stage_a_chunk(0, 0, mid_hook=lambda: kT_chunk(1))
        stage_a_chunk(2, 0)
        stage_ax(0)
        nc.sync.dma_start(xk_sb[:, 4:8, :], xk_r[:, 4:8, :])
        stage_a_chunk(2, 1, mid_hook=lambda: kT_chunk(2))
        stage_ax(2)
        nc.sync.dma_start(wv_sb[:, :, 0:512], wvp[:, :, 0:512])
        stage_a_chunk(1, 0)
        stage_a_chunk(1, 1)
        stage_twv(0, dm_half=0)
        stage_a_chunk(1, 2, mid_hook=lambda: kT_chunk(3))
        nc.sync.dma_start(wv_sb[:, :, 512:1024], wvp[:, :, 512:1024])
        stage_twv(0, dm_half=1)
        stage_a_chunk(1, 3)
        nc.sync.dma_start(xk_sb[:, 8:12, :], xk_r[:, 8:12, :])
        nc.sync.dma_start(xk_sb[:, 12:16, :], xk_r[:, 12:16, :])
        stage_twv(2, dm_half=0)
        stage_twv(2, dm_half=1)
        stage_ax(1, mid_hook=lambda: stage_a_chunk(3, 0))
        stage_twv(1, dm_half=0)
        stage_a_chunk(3, 1)
        stage_twv(1, dm_half=1)
        stage_a_chunk(3, 2)
        stage_ax(3)
        stage_twv(3, den_dma=True)

    nc.compile()
    return nc


def _get_nc():
    with _BUILD_LOCK:
        if "nc" not in _CACHE:
            _CACHE["nc"] = _build()
        return _CACHE["nc"]


def kernel(x, cross, Wq, Wk, Wv, mask):
    from concourse import bass_utils

    nc = _get_nc()

    bf = ml_dtypes.bfloat16
    x = np.asarray(x, dtype=np.float32)
    cross = np.asarray(cross, dtype=np.float32)
    scale = 1.0 / math.sqrt(DA)

    def pack_w(wT, m_cols):
        # [D, m] -> [P, KC, m] with [p, kc, m] = wT[kc*128 + p, m]
        return np.ascontiguousarray(
            wT.reshape(KC, P, m_cols).transpose(1, 0, 2)).astype(bf)

    wqp_h = pack_w((np.asarray(Wq, np.float32) * scale).T, DA)
    wkp_h = pack_w(np.asarray(Wk, np.float32).T, DA)
    wvp_h = pack_w(np.asarray(Wv, np.float32).T, D)
    mf = np.asarray(mask).astype(np.float32)  # [B, S]

    karange = np.arange(S)
    in_maps = []
    rows_per_core = []
    for core in range(NCORES):
        b, p = divmod(core, 2)
        blocks = STRIPS[p]
        rows = np.concatenate([np.arange(g * P, (g + 1) * P) for g in blocks])
        rows_per_core.append((b, rows))
        mb = mf[b]
        kneg = (-BIG * (1.0 - mb)).astype(np.float32)  # [S]
        kmb_h = np.ascontiguousarray(
            np.broadcast_to(kneg[:1536], (P, 1536))).astype(bf)
        mq = mb[rows]  # [1024]
        qmn_h = np.ascontiguousarray(
            (-BIG * (1.0 - mq)).reshape(8, P).T)  # [128, 8]
        dm_h = np.empty((8, P, 512), np.float32)
        for s, g in enumerate(blocks):
            c = PAIR_C[s // 2]
            k0 = (c - 1) * 512
            kk = karange[k0:k0 + 512]
            qq = g * P + np.arange(P)
            mqs = mq[s * P:(s + 1) * P]
            t = np.broadcast_to(kneg[k0:k0 + 512], (P, 512)).copy()
            t += -BIG * (kk[None, :] > qq[:, None])
            t += (2.0 * BIG * (1.0 - mqs))[:, None] * (kk[None, :] == qq[:, None])
            dm_h[s] = t
        in_maps.append({
            "xk": np.ascontiguousarray(x[b]).astype(bf),
            "cT": np.ascontiguousarray(cross[b].T).astype(bf),
            "xqT": np.ascontiguousarray(x[b][rows].T).astype(bf),
            "wqp": wqp_h,
            "wkp": wkp_h,
            "wvp": wvp_h,
            "kmb": kmb_h,
            "qmn": qmn_h,
            "dmask": dm_h.astype(bf),
        })

    _CACHE["in_maps"] = in_maps
    res = bass_utils.run_bass_kernel_spmd(
        nc, in_maps, core_ids=list(range(NCORES)))

    out = np.empty((B, S, D), np.float32)
    for core in range(NCORES):
        b, rows = rows_per_core[core]
        r = res.results[core]
        o = r["outT"].T  # [1024 q, 1024 dm]
        denf = r["den"].T.reshape(-1)  # [1024] strip-ordered
        out[b, rows] = o / denf[:, None]
    return out


# revision 10
# speedup vs baseline: 1.4256x; 1.1836x over previous
"""Trainium2 Bass kernel for nn_Attention_42288247996512 (sparse causal cross-attention).

reference:
  q = x @ Wq.T; k = cross @ Wk.T; v = x @ Wv.T
  logits = q @ k.T  (causal mask; padding mask m_q*m_k + eye > 0)
  out = softmax(logits / sqrt(128)) @ v

Sharding: 8 cores = 4 batches x 2 query-strips (SPMD). Each strip is 8 query
blocks (128 rows) grouped into 4 pairs of adjacent blocks.

Two structural optimizations vs a vanilla flash-style kernel:

1) Reassociation:  attn @ (x @ Wv.T) == (attn @ x) @ Wv.T.  Each core owns
   1024 query rows but would need all 2048 key rows of v, so projecting
   t = attn@x (1024 cols) instead of v (2048 rows) halves that matmul.

2) Key compaction: ~half the keys are padding-masked (exp == 0 columns).
   The kernel is JIT-specialized on the mask's *structure*: keys are
   host-compacted to the active ones, shrinking kT/logits/exp/transpose/AX
   nearly 2x.  Masked queries (whose softmax row is a delta at the diagonal,
   so out[q] = v[q]) bypass attention entirely: x.T is DMA'd into the t
   buffer and the attention results are merged over it with predicated
   copies (mask = query-unmasked), then t @ Wv.T produces v[q] for them
   directly.  The structure parameters are recomputed from the input mask on
   every call (and cached), so the kernel stays correct for any input.

All streamed operands are bf16; PSUM accumulation is f32.  Host does layout
packs/bf16 casts, gathers, additive-mask building, and the final denominator
divide + scatter (as in the baseline kernel).
"""
import math
import threading

import ml_dtypes
import numpy as np

B, S, D, DA = 4, 2048, 1024, 128
P = 128
NCORES = 8
BIG = 32768.0  # power of two: exactly representable in bf16
NQ = 1024      # query rows per core strip
KC = D // P    # 8 contraction chunks of 128

# strips: pairs of adjacent blocks; block g attends orig keys < (g+1)*128
STRIPS = [
    [0, 1, 14, 15, 6, 7, 8, 9],
    [2, 3, 12, 13, 4, 5, 10, 11],
]

_BUILD_LOCK = threading.Lock()
_CACHE: dict = {}


def _derive_params(mask_f):
    """Compute the SPMD kernel structure (max over all 8 cores) from the mask.

    Returns a hashable params tuple:
      nkb:   compacted key blocks (128 each)
      kb:    per-pair key-block count (AX contraction length)
      chunks: per-pair tuple of chunk widths (<=512, multiples of 128)
      masked: per-pair tuple of bools - does chunk j need an additive mask
    """
    nkb = 0
    kb = [0, 0, 0, 0]
    for b in range(B):
        m = mask_f[b] > 0
        ck = np.cumsum(m)          # ck[s] = # active keys <= s
        nk = int(ck[-1])
        nkb = max(nkb, (nk + P - 1) // P)
        for p in range(2):
            blocks = STRIPS[p]
            for pr in range(4):
                g = max(blocks[2 * pr], blocks[2 * pr + 1])
                bmax = int(ck[(g + 1) * P - 1])
                kb[pr] = max(kb[pr], (bmax + P - 1) // P)
    chunks = []
    for pr in range(4):
        w = kb[pr] * P
        ch = []
        while w > 0:
            ch.append(min(512, w))
            w -= min(512, w)
        chunks.append(tuple(ch))
    # chunk (pr, j) needs a mask iff for ANY core its key range reaches
    # beyond that core's (min unmasked-row boundary) or active-key count
    masked = [[False] * len(chunks[pr]) for pr in range(4)]
    for b in range(B):
        m = mask_f[b] > 0
        ck = np.cumsum(m)
        nk = int(ck[-1])
        for p in range(2):
            blocks = STRIPS[p]
            for pr in range(4):
                rows = np.concatenate(
                    [np.arange(g * P, (g + 1) * P)
                     for g in (blocks[2 * pr], blocks[2 * pr + 1])])
                urows = rows[m[rows]]
                bmin = int(ck[urows].min()) if len(urows) else 0
                base = 0
                for j, w in enumerate(chunks[pr]):
                    if base + w > bmin or base + w > nk:
                        masked[pr][j] = True
                    base += w
    return (nkb, tuple(kb), tuple(chunks),
            tuple(tuple(mj) for mj in masked))


def _build(params):
    from contextlib import ExitStack

    import concourse.bass as bass
    import concourse.mybir as mybir
    import concourse.tile as tile
    from concourse import bacc
    from concourse.masks import make_identity

    nkb, KB, CHUNKS, MASKED = params
    NK = max(nkb, 1) * P  # padded compacted key width
    nt = sum(2 * sum(mj) for mj in MASKED)  # dmask tile count
    NT = max(nt, 1)

    dt = mybir.dt
    f32 = dt.float32
    bf16 = dt.bfloat16
    AF = mybir.ActivationFunctionType
    ALU = mybir.AluOpType

    nc = bacc.Bacc("TRN2", target_bir_lowering=False, debug=False)

    # DRAM inputs (bf16 unless noted); weights host-packed to [P, ...] so
    # DMA rows are contiguous >=512B runs.
    xkc = nc.dram_tensor("xkc", [NK, D], bf16, kind="ExternalInput").ap()
    cTc = nc.dram_tensor("cTc", [D, NK], bf16, kind="ExternalInput").ap()
    xqT = nc.dram_tensor("xqT", [D, NQ], bf16, kind="ExternalInput").ap()
    wqp = nc.dram_tensor("wqp", [P, KC, DA], bf16, kind="ExternalInput").ap()
    wkp = nc.dram_tensor("wkp", [P, KC, DA], bf16, kind="ExternalInput").ap()
    wvp = nc.dram_tensor("wvp", [P, KC, D], bf16, kind="ExternalInput").ap()
    qmn = nc.dram_tensor("qmn", [P, 8], f32, kind="ExternalInput").ap()
    dm2 = nc.dram_tensor("dm2", [NT, P, 512], bf16, kind="ExternalInput").ap()
    u8 = dt.uint8
    pmask = nc.dram_tensor("pmask", [P, 4, 256], u8,
                           kind="ExternalInput").ap()

    outT = nc.dram_tensor("outT", [D, NQ], f32, kind="ExternalOutput").ap()
    den = nc.dram_tensor("den", [P, 8], f32, kind="ExternalOutput").ap()

    xkc_r = xkc.rearrange("(kb p) d -> p kb d", p=P)
    cTc_r = cTc.rearrange("(kc p) s -> p kc s", p=P)
    xqT_r = xqT.rearrange("(kc p) q -> p kc q", p=P)
    outT_r = outT.rearrange("(dmc p) q -> p dmc q", p=P)

    # dmask tile index for (pair, chunk, blk)
    dmidx = {}
    for pr in range(4):
        for j, mj in enumerate(MASKED[pr]):
            if mj:
                for blk in range(2):
                    dmidx[(pr, j, blk)] = len(dmidx)

    # kT chunk layout over NK cols
    kt_chunks = []
    w = NK
    while w > 0:
        kt_chunks.append(min(512, w))
        w -= min(512, w)

    with tile.TileContext(nc) as tc, ExitStack() as ctx:
        const = ctx.enter_context(tc.tile_pool(name="const", bufs=1))
        persist = ctx.enter_context(tc.tile_pool(name="persist", bufs=1))
        stream = ctx.enter_context(tc.tile_pool(name="stream", bufs=2))
        apool = ctx.enter_context(tc.tile_pool(name="apool", bufs=4))
        epool = ctx.enter_context(tc.tile_pool(name="epool", bufs=24))

        ident_f32 = const.tile([P, P], f32, name="ident_f32")
        make_identity(nc, ident_f32)
        ident = const.tile([P, P], bf16, name="ident")
        nc.vector.tensor_copy(ident[:], ident_f32[:])

        wq_sb = const.tile([P, KC, DA], bf16, name="wq_sb")
        wk_sb = const.tile([P, KC, DA], bf16, name="wk_sb")
        wv_sb = const.tile([P, KC, D], bf16, name="wv_sb")
        qmn_sb = const.tile([P, 8], f32, name="qmn_sb")
        dm_sb = const.tile([P, NT, 512], bf16, name="dm_sb")
        pm_sb = const.tile([P, 4, 256], u8, name="pm_sb")

        kT_sb = persist.tile([P, NK], bf16, name="kT_sb")
        qT_sb = persist.tile([P, NQ], bf16, name="qT_sb")
        xk_sb = persist.tile([P, max(nkb, 1), D], bf16, name="xk_sb")
        tT_sb = persist.tile([P, KC, NQ], bf16, name="tT_sb")
        den_sb = persist.tile([P, 8], f32, name="den_sb")

        eTs_all = {pr: [] for pr in range(4)}
        daccs_all = {pr: [[], []] for pr in range(4)}

        # PSUM: 4 pools x 2 bufs x 2KB = all 8 banks.
        psl_pool = ctx.enter_context(
            tc.tile_pool(name="psl", bufs=2, space="PSUM"))
        psT_pool = ctx.enter_context(
            tc.tile_pool(name="psT", bufs=2, space="PSUM"))
        psax_pool = ctx.enter_context(
            tc.tile_pool(name="psax", bufs=2, space="PSUM"))
        psw_pool = ctx.enter_context(
            tc.tile_pool(name="psw", bufs=2, space="PSUM"))

        # ---- projections (DMA emission order == SP FIFO delivery order) ----
        def kT_chunk(j):
            if nkb == 0:
                return
            w = kt_chunks[j]
            base = sum(kt_chunks[:j])
            ctj = stream.tile([P, KC, 512], bf16, tag="ct", name=f"ct{j}",
                              bufs=2)
            nc.sync.dma_start(ctj[:, :, :w], cTc_r[:, :, base:base + w])
            ps_k = psax_pool.tile([P, 512], f32, tag="psax", name=f"ps_k{j}")
            for kc in range(KC):
                nc.tensor.matmul(
                    ps_k[:, :w],
                    lhsT=wk_sb[:, kc, :],
                    rhs=ctj[:, kc, :w],
                    start=(kc == 0), stop=(kc == KC - 1),
                )
            nc.any.tensor_copy(kT_sb[:, base:base + w], ps_k[:, :w])

        nc.sync.dma_start(wq_sb[:], wqp)
        ps_q = [psl_pool.tile([P, 512], f32, tag="psl", name=f"ps_q{n}")
                for n in range(2)]

        def qT_chunk(kc):
            xqt = stream.tile([P, NQ], bf16, tag="xq", name=f"xq{kc}", bufs=3)
            nc.sync.dma_start(xqt[:], xqT_r[:, kc, :])
            for n in range(2):
                nc.tensor.matmul(
                    ps_q[n][:],
                    lhsT=wq_sb[:, kc, :],
                    rhs=xqt[:, n * 512:(n + 1) * 512],
                    start=(kc == 0), stop=(kc == KC - 1),
                )

        qT_chunk(0)
        nc.sync.dma_start(wk_sb[:], wkp)
        kT_chunk(0)
        for kc in range(1, KC):
            qT_chunk(kc)
        for n in range(2):
            nc.any.tensor_copy(qT_sb[:, n * 512:(n + 1) * 512], ps_q[n][:])
        nc.sync.dma_start(qmn_sb[:], qmn[:])
        nc.sync.dma_start(dm_sb[:], dm2.rearrange("t p w -> p t w"))
        nc.sync.dma_start(pm_sb[:], pmask)
        if nkb:
            nc.sync.dma_start(xk_sb[:, 0:min(KB[0], nkb), :],
                              xkc_r[:, 0:min(KB[0], nkb), :])
        # pass-through: t.T defaults to x_strip.T so masked queries produce
        # out[q] = v[q]; attention results are merged in with predicated
        # copies below.
        nc.sync.dma_start(tT_sb[:], xqT_r)

        # ---- attention stages ----
        def stage_a_chunk(pr, j, mid_hook=None):
            w = CHUNKS[pr][j]
            base = sum(CHUNKS[pr][:j])
            nks = (w + P - 1) // P
            psTs = [psT_pool.tile([P, 256], bf16, tag="psT",
                                  name=f"psT{pr}_{j}_{ks}",
                                  padded_shape=[P, 1024])
                    for ks in range(nks)]
            es = []
            for blk in range(2):
                slot = pr * 2 + blk
                psl = psl_pool.tile([P, 512], f32, tag="psl",
                                    name=f"psl{slot}_{j}")
                nc.tensor.matmul(
                    psl[:, :w],
                    lhsT=qT_sb[:, slot * P:(slot + 1) * P],
                    rhs=kT_sb[:, base:base + w],
                    start=True, stop=True,
                )
                dac = apool.tile([P, 1], f32, tag="dac",
                                 name=f"dac{slot}_{j}", bufs=12)
                e = apool.tile([P, 512], bf16, tag="e", name=f"e{slot}_{j}")
                if MASKED[pr][j]:
                    sbl = apool.tile([P, 512], f32, tag="sbl",
                                     name=f"sbl{slot}_{j}")
                    nc.vector.tensor_tensor(
                        out=sbl[:, :w], in0=psl[:, :w],
                        in1=dm_sb[:, dmidx[(pr, j, blk)], :w], op=ALU.add)
                    src = sbl
                else:
                    src = psl
                nc.scalar.activation(
                    e[:, :w], src[:, :w], AF.Exp,
                    bias=qmn_sb[:, slot:slot + 1], scale=1.0,
                    accum_out=dac[:],
                )
                daccs_all[pr][blk].append(dac)
                es.append(e)
            if mid_hook is not None:
                mid_hook()
            for blk in range(2):
                for ks in range(nks):
                    nc.tensor.transpose(
                        psTs[ks][:, blk * P:(blk + 1) * P],
                        es[blk][:, ks * P:(ks + 1) * P],
                        ident[:],
                    )
            for ks in range(nks):
                eT = epool.tile([P, 256], bf16, tag="eT",
                                name=f"eT{pr}_{j}_{ks}")
                nc.any.tensor_copy(eT[:], psTs[ks][:])
                eTs_all[pr].append(eT)

        def stage_den(pr):
            for blk in range(2):
                slot = pr * 2 + blk
                dl = daccs_all[pr][blk]
                dst = den_sb[:, slot:slot + 1]
                if len(dl) == 1:
                    nc.any.tensor_copy(dst, dl[0][:])
                else:
                    nc.vector.tensor_tensor(
                        out=dst, in0=dl[0][:], in1=dl[1][:], op=ALU.add)
                    for d in dl[2:]:
                        nc.vector.tensor_tensor(
                            out=dst, in0=dst, in1=d[:], op=ALU.add)

        def stage_ax(pr, mid_hook=None):
            # t.T[xd, q] = sum_kb x[kb].T-contraction with eT over the
            # pair's compacted key blocks; merged into the pass-through
            # with a predicated copy (pmask: 1 = query unmasked).
            stage_den(pr)
            eTs = eTs_all[pr]
            for xdc in range(KC):
                psax = psax_pool.tile([P, 256], f32, tag="psax",
                                      name=f"psax{pr}_{xdc}",
                                      padded_shape=[P, 512])
                for kb in range(KB[pr]):
                    nc.tensor.matmul(
                        psax[:],
                        lhsT=xk_sb[:, kb, xdc * P:(xdc + 1) * P],
                        rhs=eTs[kb][:],
                        start=(kb == 0), stop=(kb == KB[pr] - 1),
                    )
                nc.vector.copy_predicated(
                    tT_sb[:, xdc, pr * 256:(pr + 1) * 256],
                    pm_sb[:, pr, :], psax[:])
                if mid_hook is not None and xdc == 3:
                    mid_hook()

        _osbs = {}

        def stage_twv(pr, dm_half=None, den_dma=False):
            # out.T[dm, q] = Wv.T.T @ t.T for this pair's 256 query columns
            halves = [0, 1] if dm_half is None else [dm_half]
            osb = _osbs.setdefault(
                pr, apool.tile([P, KC, 256], f32, tag="osb",
                               name=f"osb{pr}", bufs=2))
            if den_dma:
                nc.sync.dma_start(den[:], den_sb[:])
            for h in halves:
                for dmc in range(4 * h, 4 * h + 4):
                    psw = psw_pool.tile([P, 256], f32, tag="psw",
                                        name=f"psw{pr}_{dmc}",
                                        padded_shape=[P, 512])
                    for xdc in range(KC):
                        nc.tensor.matmul(
                            psw[:],
                            lhsT=wv_sb[:, xdc, dmc * P:(dmc + 1) * P],
                            rhs=tT_sb[:, xdc, pr * 256:(pr + 1) * 256],
                            start=(xdc == 0), stop=(xdc == KC - 1),
                        )
                    # alternate engines so the final copies drain in parallel
                    if dmc % 2 == 0:
                        nc.vector.tensor_copy(osb[:, dmc, :], psw[:])
                    else:
                        nc.scalar.copy(osb[:, dmc, :], psw[:])
                nc.sync.dma_start(
                    outT_r[:, 4 * h:4 * h + 4, pr * 256:(pr + 1) * 256],
                    osb[:, 4 * h:4 * h + 4, :])

        def A(pr, j, mid_hook=None):
            if j < len(CHUNKS[pr]) and CHUNKS[pr][j] > 0:
                stage_a_chunk(pr, j, mid_hook=mid_hook)
            elif mid_hook is not None:
                mid_hook()

        def AX(pr, mid_hook=None):
            if KB[pr] > 0:
                stage_ax(pr, mid_hook=mid_hook)
            elif mid_hook is not None:
                mid_hook()

        # ---- fused schedule (PE emission order tuned to DMA arrivals) ----
        nch = [len(CHUNKS[pr]) for pr in range(4)]
        A(0, 0, mid_hook=lambda: kT_chunk(1) if len(kt_chunks) > 1 else None)
        AX(0)
        if nkb:
            nc.sync.dma_start(xk_sb[:, min(KB[0], nkb):nkb, :],
                              xkc_r[:, min(KB[0], nkb):nkb, :])
        A(2, 0, mid_hook=lambda: [kT_chunk(j) for j in
                                  range(2, len(kt_chunks))])
        A(2, 1)
        AX(2)
        nc.sync.dma_start(wv_sb[:, :, 0:512], wvp[:, :, 0:512])
        for j in range(nch[1]):
            A(1, j)
        nc.sync.dma_start(wv_sb[:, :, 512:1024], wvp[:, :, 512:1024])
        A(3, 0)
        AX(1)
        stage_twv(0, dm_half=0)
        A(3, 1)
        stage_twv(0, dm_half=1)
        for j in range(2, nch[3]):
            A(3, j)
        stage_twv(2)
        AX(3)
        stage_twv(1)
        stage_twv(3, den_dma=True)

    nc.compile()
    return nc


def _get_nc(params=None):
    with _BUILD_LOCK:
        if params is None:
            # harness/test introspection path: last-built (or default) kernel
            if "nc" in _CACHE:
                return _CACHE["nc"]
            params = _CACHE.get("params")
            if params is None:
                raise RuntimeError("call kernel() first to JIT the program")
        if _CACHE.get("params") != params or "nc" not in _CACHE:
            _CACHE["params"] = params
            _CACHE["nc"] = _build(params)
        return _CACHE["nc"]


def kernel(x, cross, Wq, Wk, Wv, mask):
    from concourse import bass_utils

    bf = ml_dtypes.bfloat16
    x = np.asarray(x, dtype=np.float32)
    cross = np.asarray(cross, dtype=np.float32)
    scale = 1.0 / math.sqrt(DA)
    mf = np.asarray(mask).astype(np.float32)  # [B, S]

    params = _derive_params(mf)
    nc = _get_nc(params)
    nkb, KB, CHUNKS, MASKED = params
    NK = max(nkb, 1) * P
    NT = max(sum(2 * sum(mj) for mj in MASKED), 1)

    def pack_w(wT, m_cols):
        # [D, m] -> [P, KC, m] with [p, kc, m] = wT[kc*128 + p, m]
        return np.ascontiguousarray(
            wT.reshape(KC, P, m_cols).transpose(1, 0, 2)).astype(bf)

    wqp_h = pack_w((np.asarray(Wq, np.float32) * scale).T, DA)
    wkp_h = pack_w(np.asarray(Wk, np.float32).T, DA)
    wvp_h = pack_w(np.asarray(Wv, np.float32).T, D)

    in_maps = []
    rows_per_core = []
    for core in range(NCORES):
        b, p = divmod(core, 2)
        blocks = STRIPS[p]
        rows = np.concatenate([np.arange(g * P, (g + 1) * P) for g in blocks])
        mb = mf[b] > 0
        ck = np.cumsum(mb)           # active keys <= s
        active = np.nonzero(mb)[0]   # orig idx of compacted keys
        nk = len(active)
        rows_per_core.append((b, rows, mb[rows]))
        # compacted key-side tensors (zero pad to NK)
        xkc_h = np.zeros((NK, D), np.float32)
        xkc_h[:nk] = x[b][active]
        cTc_h = np.zeros((D, NK), np.float32)
        cTc_h[:, :nk] = cross[b].T[:, active]
        mq = mb[rows]
        qmn_h = np.ascontiguousarray(
            (-BIG * (1.0 - mq.astype(np.float32))).reshape(8, P).T)
        # additive causal/pad masks in compacted key coords, per masked chunk
        dm_h = np.full((NT, P, 512), -BIG, np.float32)
        ck_rows = ck[rows]  # allowed-key count per strip row
        ti = 0
        for pr in range(4):
            base = 0
            for j, w in enumerate(CHUNKS[pr]):
                if MASKED[pr][j]:
                    for blk in range(2):
                        ckb = ck_rows[(pr * 2 + blk) * P:
                                      (pr * 2 + blk + 1) * P]
                        kidx = base + np.arange(w)
                        dm_h[ti, :, :w] = np.where(
                            kidx[None, :] < ckb[:, None], 0.0, -BIG)
                        ti += 1
                base += w
        # predication mask: 1 = query unmasked (take AX result)
        pm_h = np.broadcast_to(
            mq.astype(np.float32).reshape(4, 256)[None, :, :], (P, 4, 256))
        in_maps.append({
            "xkc": xkc_h.astype(bf),
            "cTc": cTc_h.astype(bf),
            "xqT": np.ascontiguousarray(x[b][rows].T).astype(bf),
            "wqp": wqp_h,
            "wkp": wkp_h,
            "wvp": wvp_h,
            "qmn": qmn_h,
            "dm2": dm_h.astype(bf),
            "pmask": np.ascontiguousarray(pm_h).astype(np.uint8),
        })

    _CACHE["in_maps"] = in_maps
    res = bass_utils.run_bass_kernel_spmd(
        nc, in_maps, core_ids=list(range(NCORES)))

    out = np.empty((B, S, D), np.float32)
    for core in range(NCORES):
        b, rows, mq = rows_per_core[core]
        r = res.results[core]
        o = r["outT"].T  # [1024 q, 1024 dm]
        denf = r["den"].T.reshape(-1)  # [1024] strip-ordered
        denf = np.where(mq, denf, 1.0)  # masked queries: out = v[q] directly
        out[b, rows] = o / denf[:, None]
    return out


# revision 12
# speedup vs baseline: 1.5475x; 1.0855x over previous
"""Trainium2 Bass kernel for nn_Attention_42288247996512 (sparse causal cross-attention).

reference:
  q = x @ Wq.T; k = cross @ Wk.T; v = x @ Wv.T
  logits = q @ k.T  (causal mask; padding mask m_q*m_k + eye > 0)
  out = softmax(logits / sqrt(128)) @ v

Sharding: 8 cores = 4 batches x 2 query-strips (SPMD). Each strip is 8 query
blocks (128 rows) grouped into 4 pairs of adjacent blocks.

Two structural optimizations vs a vanilla flash-style kernel:

1) Reassociation:  attn @ (x @ Wv.T) == (attn @ x) @ Wv.T.  Each core owns
   1024 query rows but would need all 2048 key rows of v, so projecting
   t = attn@x (1024 cols) instead of v (2048 rows) halves that matmul.

2) Key compaction: ~half the keys are padding-masked (exp == 0 columns).
   The kernel is JIT-specialized on the mask's *structure*: keys are
   host-compacted to the active ones, shrinking kT/logits/exp/transpose/AX
   nearly 2x.  Masked queries (whose softmax row is a delta at the diagonal,
   so out[q] = v[q]) bypass attention entirely: x.T is DMA'd into the t
   buffer and the attention results are merged over it with predicated
   copies (mask = query-unmasked), then t @ Wv.T produces v[q] for them
   directly.  The structure parameters are recomputed from the input mask on
   every call (and cached), so the kernel stays correct for any input.

All streamed operands are bf16; PSUM accumulation is f32.  Host does layout
packs/bf16 casts, gathers, additive-mask building, and the final denominator
divide + scatter (as in the baseline kernel).
"""
import math
import threading

import ml_dtypes
import numpy as np

B, S, D, DA = 4, 2048, 1024, 128
P = 128
NCORES = 8
BIG = 32768.0  # power of two: exactly representable in bf16
NQ = 1024      # query rows per core strip
KC = D // P    # 8 contraction chunks of 128

# strips: pairs of adjacent blocks; block g attends orig keys < (g+1)*128
STRIPS = [
    [0, 1, 14, 15, 6, 7, 8, 9],
    [2, 3, 12, 13, 4, 5, 10, 11],
]

_BUILD_LOCK = threading.Lock()
_CACHE: dict = {}


def _derive_params(mask_f):
    """Compute the SPMD kernel structure (max over all 8 cores) from the mask.

    Returns a hashable params tuple:
      nkb:   compacted key blocks (128 each)
      kb:    per-pair key-block count (AX contraction length)
      chunks: per-pair tuple of chunk widths (<=512, multiples of 128)
      masked: per-pair tuple of bools - does chunk j need an additive mask
    """
    nkb = 0
    kb = [0, 0, 0, 0]
    for b in range(B):
        m = mask_f[b] > 0
        ck = np.cumsum(m)          # ck[s] = # active keys <= s
        nk = int(ck[-1])
        nkb = max(nkb, (nk + P - 1) // P)
        for p in range(2):
            blocks = STRIPS[p]
            for pr in range(4):
                g = max(blocks[2 * pr], blocks[2 * pr + 1])
                bmax = int(ck[(g + 1) * P - 1])
                kb[pr] = max(kb[pr], (bmax + P - 1) // P)
    chunks = []
    for pr in range(4):
        w = kb[pr] * P
        ch = []
        while w > 0:
            ch.append(min(512, w))
            w -= min(512, w)
        chunks.append(tuple(ch))
    # chunk (pr, j) needs a mask iff for ANY core its key range reaches
    # beyond that core's (min unmasked-row boundary) or active-key count
    masked = [[False] * len(chunks[pr]) for pr in range(4)]
    for b in range(B):
        m = mask_f[b] > 0
        ck = np.cumsum(m)
        nk = int(ck[-1])
        for p in range(2):
            blocks = STRIPS[p]
            for pr in range(4):
                rows = np.concatenate(
                    [np.arange(g * P, (g + 1) * P)
                     for g in (blocks[2 * pr], blocks[2 * pr + 1])])
                urows = rows[m[rows]]
                bmin = int(ck[urows].min()) if len(urows) else 0
                base = 0
                for j, w in enumerate(chunks[pr]):
                    if base + w > bmin or base + w > nk:
                        masked[pr][j] = True
                    base += w
    return (nkb, tuple(kb), tuple(chunks),
            tuple(tuple(mj) for mj in masked))


def _build(params):
    from contextlib import ExitStack

    import concourse.bass as bass
    import concourse.mybir as mybir
    import concourse.tile as tile
    from concourse import bacc
    from concourse.masks import make_identity

    nkb, KB, CHUNKS, MASKED = params
    NK = max(nkb, 1) * P  # padded compacted key width
    nt = sum(2 * sum(mj) for mj in MASKED)  # dmask tile count
    NT = max(nt, 1)

    dt = mybir.dt
    f32 = dt.float32
    bf16 = dt.bfloat16
    AF = mybir.ActivationFunctionType
    ALU = mybir.AluOpType

    nc = bacc.Bacc("TRN2", target_bir_lowering=False, debug=False)

    # DRAM inputs (bf16 unless noted); weights host-packed to [P, ...] so
    # DMA rows are contiguous >=512B runs.
    xkc = nc.dram_tensor("xkc", [NK, D], bf16, kind="ExternalInput").ap()
    cTc = nc.dram_tensor("cTc", [D, NK], bf16, kind="ExternalInput").ap()
    xqT = nc.dram_tensor("xqT", [D, NQ], bf16, kind="ExternalInput").ap()
    wqp = nc.dram_tensor("wqp", [P, KC, DA], bf16, kind="ExternalInput").ap()
    wkp = nc.dram_tensor("wkp", [P, KC, DA], bf16, kind="ExternalInput").ap()
    wvp = nc.dram_tensor("wvp", [P, KC, D], bf16, kind="ExternalInput").ap()
    qmn = nc.dram_tensor("qmn", [P, 8], f32, kind="ExternalInput").ap()
    dm2 = nc.dram_tensor("dm2", [NT, P, 512], bf16, kind="ExternalInput").ap()
    u8 = dt.uint8
    pmask = nc.dram_tensor("pmask", [P, 4, 256], u8,
                           kind="ExternalInput").ap()

    outT = nc.dram_tensor("outT", [D, NQ], f32, kind="ExternalOutput").ap()
    den = nc.dram_tensor("den", [P, 8], f32, kind="ExternalOutput").ap()

    xkc_r = xkc.rearrange("(kb p) d -> p kb d", p=P)
    cTc_r = cTc.rearrange("(kc p) s -> p kc s", p=P)
    xqT_r = xqT.rearrange("(kc p) q -> p kc q", p=P)
    outT_r = outT.rearrange("(dmc p) q -> p dmc q", p=P)

    # dmask tile index for (pair, chunk, blk): assigned lazily in emission
    # order so the DMA (split in two) streams tiles in first-use order; the
    # host builds dm2 in this same order (read back via _CACHE["dm_order"]).
    dmidx = {}

    def dm_tile(pr, j, blk):
        key = (pr, j, blk)
        if key not in dmidx:
            dmidx[key] = len(dmidx)
        return dmidx[key]

    # kT chunk layout over NK cols
    kt_chunks = []
    w = NK
    while w > 0:
        kt_chunks.append(min(512, w))
        w -= min(512, w)

    with tile.TileContext(nc) as tc, ExitStack() as ctx:
        const = ctx.enter_context(tc.tile_pool(name="const", bufs=1))
        persist = ctx.enter_context(tc.tile_pool(name="persist", bufs=1))
        stream = ctx.enter_context(tc.tile_pool(name="stream", bufs=2))
        apool = ctx.enter_context(tc.tile_pool(name="apool", bufs=4))
        epool = ctx.enter_context(tc.tile_pool(name="epool", bufs=24))

        ident_f32 = const.tile([P, P], f32, name="ident_f32")
        make_identity(nc, ident_f32)
        ident = const.tile([P, P], bf16, name="ident")
        nc.vector.tensor_copy(ident[:], ident_f32[:])

        wq_sb = const.tile([P, KC, DA], bf16, name="wq_sb")
        wk_sb = const.tile([P, KC, DA], bf16, name="wk_sb")
        wv_sb = const.tile([P, KC, D], bf16, name="wv_sb")
        qmn_sb = const.tile([P, 8], f32, name="qmn_sb")
        dm_sb = const.tile([P, NT, 512], bf16, name="dm_sb")
        pm_sb = const.tile([P, 4, 256], u8, name="pm_sb")

        kT_sb = persist.tile([P, NK], bf16, name="kT_sb")
        qT_sb = persist.tile([P, NQ], bf16, name="qT_sb")
        xk_sb = persist.tile([P, max(nkb, 1), D], bf16, name="xk_sb")
        tT_sb = persist.tile([P, KC, NQ], bf16, name="tT_sb")
        den_sb = persist.tile([P, 8], f32, name="den_sb")

        eTs_all = {pr: [] for pr in range(4)}
        daccs_all = {pr: [[], []] for pr in range(4)}

        # PSUM: 4 pools x 2 bufs x 2KB = all 8 banks.
        psl_pool = ctx.enter_context(
            tc.tile_pool(name="psl", bufs=2, space="PSUM"))
        psT_pool = ctx.enter_context(
            tc.tile_pool(name="psT", bufs=2, space="PSUM"))
        psax_pool = ctx.enter_context(
            tc.tile_pool(name="psax", bufs=2, space="PSUM"))
        psw_pool = ctx.enter_context(
            tc.tile_pool(name="psw", bufs=2, space="PSUM"))

        # ---- projections (DMA emission order == SP FIFO delivery order) ----
        def kT_chunk(j):
            if nkb == 0:
                return
            w = kt_chunks[j]
            base = sum(kt_chunks[:j])
            ctj = stream.tile([P, KC, 512], bf16, tag="ct", name=f"ct{j}",
                              bufs=2)
            nc.sync.dma_start(ctj[:, :, :w], cTc_r[:, :, base:base + w])
            ps_k = psax_pool.tile([P, 512], f32, tag="psax", name=f"ps_k{j}")
            for kc in range(KC):
                nc.tensor.matmul(
                    ps_k[:, :w],
                    lhsT=wk_sb[:, kc, :],
                    rhs=ctj[:, kc, :w],
                    start=(kc == 0), stop=(kc == KC - 1),
                )
            nc.any.tensor_copy(kT_sb[:, base:base + w], ps_k[:, :w])

        # t.T is pre-filled with x_strip.T: it doubles as the qT projection
        # rhs AND as the pass-through giving masked queries out[q] = v[q]
        # (attention results are merged over it with predicated copies).
        nc.sync.dma_start(wq_sb[:], wqp)
        ps_q = [psl_pool.tile([P, 512], f32, tag="psl", name=f"ps_q{n}")
                for n in range(2)]

        def qT_half(n):
            nc.sync.dma_start(tT_sb[:, :, n * 512:(n + 1) * 512],
                              xqT_r[:, :, n * 512:(n + 1) * 512])
            for kc in range(KC):
                nc.tensor.matmul(
                    ps_q[n][:],
                    lhsT=wq_sb[:, kc, :],
                    rhs=tT_sb[:, kc, n * 512:(n + 1) * 512],
                    start=(kc == 0), stop=(kc == KC - 1),
                )

        qT_half(0)
        nc.sync.dma_start(wk_sb[:], wkp)
        kT_chunk(0)
        qT_half(1)
        for n in range(2):
            nc.any.tensor_copy(qT_sb[:, n * 512:(n + 1) * 512], ps_q[n][:])
        nc.sync.dma_start(qmn_sb[:], qmn[:])
        NTA = min(6, NT)
        nc.sync.dma_start(dm_sb[:, 0:NTA, :],
                          dm2.rearrange("t p w -> p t w")[:, 0:NTA, :])
        nc.sync.dma_start(pm_sb[:], pmask)

        # ---- attention stages ----
        def stage_a_chunk(pr, j, mid_hook=None):
            w = CHUNKS[pr][j]
            base = sum(CHUNKS[pr][:j])
            nks = (w + P - 1) // P
            psTs = [psT_pool.tile([P, 256], bf16, tag="psT",
                                  name=f"psT{pr}_{j}_{ks}",
                                  padded_shape=[P, 1024])
                    for ks in range(nks)]
            es = []
            for blk in range(2):
                slot = pr * 2 + blk
                psl = psl_pool.tile([P, 512], f32, tag="psl",
                                    name=f"psl{slot}_{j}")
                nc.tensor.matmul(
                    psl[:, :w],
                    lhsT=qT_sb[:, slot * P:(slot + 1) * P],
                    rhs=kT_sb[:, base:base + w],
                    start=True, stop=True,
                )
                dac = apool.tile([P, 1], f32, tag="dac",
                                 name=f"dac{slot}_{j}", bufs=12)
                e = apool.tile([P, 512], bf16, tag="e", name=f"e{slot}_{j}")
                if MASKED[pr][j]:
                    sbl = apool.tile([P, 512], f32, tag="sbl",
                                     name=f"sbl{slot}_{j}")
                    nc.vector.tensor_tensor(
                        out=sbl[:, :w], in0=psl[:, :w],
                        in1=dm_sb[:, dm_tile(pr, j, blk), :w], op=ALU.add)
                    src = sbl
                else:
                    src = psl
                nc.scalar.activation(
                    e[:, :w], src[:, :w], AF.Exp,
                    bias=qmn_sb[:, slot:slot + 1], scale=1.0,
                    accum_out=dac[:],
                )
                daccs_all[pr][blk].append(dac)
                es.append(e)
            if mid_hook is not None:
                mid_hook()
            for blk in range(2):
                for ks in range(nks):
                    nc.tensor.transpose(
                        psTs[ks][:, blk * P:(blk + 1) * P],
                        es[blk][:, ks * P:(ks + 1) * P],
                        ident[:],
                    )
            for ks in range(nks):
                eT = epool.tile([P, 256], bf16, tag="eT",
                                name=f"eT{pr}_{j}_{ks}")
                nc.any.tensor_copy(eT[:], psTs[ks][:])
                eTs_all[pr].append(eT)

        def stage_den(pr):
            for blk in range(2):
                slot = pr * 2 + blk
                dl = daccs_all[pr][blk]
                dst = den_sb[:, slot:slot + 1]
                if len(dl) == 1:
                    nc.any.tensor_copy(dst, dl[0][:])
                else:
                    nc.vector.tensor_tensor(
                        out=dst, in0=dl[0][:], in1=dl[1][:], op=ALU.add)
                    for d in dl[2:]:
                        nc.vector.tensor_tensor(
                            out=dst, in0=dst, in1=d[:], op=ALU.add)

        def stage_ax(pr, mid_hook=None):
            # t.T[xd, q] = sum_kb x[kb].T-contraction with eT over the
            # pair's compacted key blocks; merged into the pass-through
            # with a predicated copy (pmask: 1 = query unmasked).
            stage_den(pr)
            eTs = eTs_all[pr]
            for xdc in range(KC):
                psax = psax_pool.tile([P, 256], f32, tag="psax",
                                      name=f"psax{pr}_{xdc}",
                                      padded_shape=[P, 512])
                for kb in range(KB[pr]):
                    nc.tensor.matmul(
                        psax[:],
                        lhsT=xk_sb[:, kb, xdc * P:(xdc + 1) * P],
                        rhs=eTs[kb][:],
                        start=(kb == 0), stop=(kb == KB[pr] - 1),
                    )
                nc.vector.copy_predicated(
                    tT_sb[:, xdc, pr * 256:(pr + 1) * 256],
                    pm_sb[:, pr, :], psax[:])
                if mid_hook is not None and xdc == 3:
                    mid_hook()

        _osbs = {}

        def stage_twv(pr, dm_half=None, den_dma=False):
            # out.T[dm, q] = Wv.T.T @ t.T for this pair's 256 query columns
            halves = [0, 1] if dm_half is None else [dm_half]
            osb = _osbs.setdefault(
                pr, apool.tile([P, KC, 256], f32, tag="osb",
                               name=f"osb{pr}", bufs=2))
            if den_dma:
                nc.sync.dma_start(den[:], den_sb[:])
            for h in halves:
                for dmc in range(4 * h, 4 * h + 4):
                    psw = psw_pool.tile([P, 256], f32, tag="psw",
                                        name=f"psw{pr}_{dmc}",
                                        padded_shape=[P, 512])
                    for xdc in range(KC):
                        nc.tensor.matmul(
                            psw[:],
                            lhsT=wv_sb[:, xdc, dmc * P:(dmc + 1) * P],
                            rhs=tT_sb[:, xdc, pr * 256:(pr + 1) * 256],
                            start=(xdc == 0), stop=(xdc == KC - 1),
                        )
                    # alternate engines so the final copies drain in parallel
                    if dmc % 2 == 0:
                        nc.vector.tensor_copy(osb[:, dmc, :], psw[:])
                    else:
                        nc.scalar.copy(osb[:, dmc, :], psw[:])
                nc.sync.dma_start(
                    outT_r[:, 4 * h:4 * h + 4, pr * 256:(pr + 1) * 256],
                    osb[:, 4 * h:4 * h + 4, :])

        def A(pr, j, mid_hook=None):
            if j < len(CHUNKS[pr]) and CHUNKS[pr][j] > 0:
                stage_a_chunk(pr, j, mid_hook=mid_hook)
            elif mid_hook is not None:
                mid_hook()

        def AX(pr, mid_hook=None):
            if KB[pr] > 0:
                stage_ax(pr, mid_hook=mid_hook)
            elif mid_hook is not None:
                mid_hook()

        # ---- fused schedule (PE emission order tuned to DMA arrivals) ----
        nch = [len(CHUNKS[pr]) for pr in range(4)]
        ka = min(KB[0], nkb)
        kbb = min(max(KB[0], KB[2]), nkb)
        A(0, 0)
        # A(1,0) is the mask-free chunk: cheap PE filler while masks stream
        if nch[1] > 0:
            A(1, 0, mid_hook=lambda: kT_chunk(1)
              if len(kt_chunks) > 1 else None)
        if nch[1] > 1:
            A(1, 1)
        if nkb and ka:
            nc.sync.dma_start(xk_sb[:, 0:ka, :], xkc_r[:, 0:ka, :])
        AX(0)
        if NT > NTA:
            nc.sync.dma_start(dm_sb[:, NTA:NT, :],
                              dm2.rearrange("t p w -> p t w")[:, NTA:NT, :])
        A(2, 0, mid_hook=lambda: [kT_chunk(j) for j in
                                  range(2, len(kt_chunks))])
        A(2, 1)
        if nkb and kbb > ka:
            nc.sync.dma_start(xk_sb[:, ka:kbb, :], xkc_r[:, ka:kbb, :])
        AX(2)
        for j in range(2, nch[1]):
            A(1, j)
        if nkb and nkb > kbb:
            nc.sync.dma_start(xk_sb[:, kbb:nkb, :], xkc_r[:, kbb:nkb, :])
        nc.sync.dma_start(wv_sb[:, :, 0:512], wvp[:, :, 0:512])
        A(3, 0)
        AX(1)
        nc.sync.dma_start(wv_sb[:, :, 512:1024], wvp[:, :, 512:1024])
        stage_twv(0, dm_half=0)
        A(3, 1)
        stage_twv(0, dm_half=1)
        for j in range(2, nch[3]):
            A(3, j)
        stage_twv(2)
        AX(3)
        stage_twv(1)
        stage_twv(3, den_dma=True)

    nc.compile()
    return nc, dmidx


def _get_nc(params=None):
    with _BUILD_LOCK:
        if params is None:
            # harness/test introspection path: last-built (or default) kernel
            if "nc" in _CACHE:
                return _CACHE["nc"]
            params = _CACHE.get("params")
            if params is None:
                raise RuntimeError("call kernel() first to JIT the program")
        if _CACHE.get("params") != params or "nc" not in _CACHE:
            _CACHE["params"] = params
            _CACHE["nc"], _CACHE["dm_order"] = _build(params)
        return _CACHE["nc"]


def kernel(x, cross, Wq, Wk, Wv, mask):
    from concourse import bass_utils

    bf = ml_dtypes.bfloat16
    x = np.asarray(x, dtype=np.float32)
    cross = np.asarray(cross, dtype=np.float32)
    scale = 1.0 / math.sqrt(DA)
    mf = np.asarray(mask).astype(np.float32)  # [B, S]

    params = _derive_params(mf)
    nc = _get_nc(params)
    nkb, KB, CHUNKS, MASKED = params
    NK = max(nkb, 1) * P
    NT = max(sum(2 * sum(mj) for mj in MASKED), 1)

    def pack_w(wT, m_cols):
        # [D, m] -> [P, KC, m] with [p, kc, m] = wT[kc*128 + p, m]
        return np.ascontiguousarray(
            wT.reshape(KC, P, m_cols).transpose(1, 0, 2)).astype(bf)

    wqp_h = pack_w((np.asarray(Wq, np.float32) * scale).T, DA)
    wkp_h = pack_w(np.asarray(Wk, np.float32).T, DA)
    wvp_h = pack_w(np.asarray(Wv, np.float32).T, D)

    in_maps = []
    rows_per_core = []
    for core in range(NCORES):
        b, p = divmod(core, 2)
        blocks = STRIPS[p]
        rows = np.concatenate([np.arange(g * P, (g + 1) * P) for g in blocks])
        mb = mf[b] > 0
        ck = np.cumsum(mb)           # active keys <= s
        active = np.nonzero(mb)[0]   # orig idx of compacted keys
        nk = len(active)
        rows_per_core.append((b, rows, mb[rows]))
        # compacted key-side tensors (zero pad to NK)
        xkc_h = np.zeros((NK, D), np.float32)
        xkc_h[:nk] = x[b][active]
        cTc_h = np.zeros((D, NK), np.float32)
        cTc_h[:, :nk] = cross[b].T[:, active]
        mq = mb[rows]
        qmn_h = np.ascontiguousarray(
            (-BIG * (1.0 - mq.astype(np.float32))).reshape(8, P).T)
        # additive causal/pad masks in compacted key coords, per masked chunk
        dm_h = np.full((NT, P, 512), -BIG, np.float32)
        ck_rows = ck[rows]  # allowed-key count per strip row
        for (pr, j, blk), ti in _CACHE["dm_order"].items():
            w = CHUNKS[pr][j]
            base = sum(CHUNKS[pr][:j])
            ckb = ck_rows[(pr * 2 + blk) * P:(pr * 2 + blk + 1) * P]
            kidx = base + np.arange(w)
            dm_h[ti, :, :w] = np.where(
                kidx[None, :] < ckb[:, None], 0.0, -BIG)
        # predication mask: 1 = query unmasked (take AX result)
        pm_h = np.broadcast_to(
            mq.astype(np.float32).reshape(4, 256)[None, :, :], (P, 4, 256))
        in_maps.append({
            "xkc": xkc_h.astype(bf),
            "cTc": cTc_h.astype(bf),
            "xqT": np.ascontiguousarray(x[b][rows].T).astype(bf),
            "wqp": wqp_h,
            "wkp": wkp_h,
            "wvp": wvp_h,
            "qmn": qmn_h,
            "dm2": dm_h.astype(bf),
            "pmask": np.ascontiguousarray(pm_h).astype(np.uint8),
        })

    _CACHE["in_maps"] = in_maps
    res = bass_utils.run_bass_kernel_spmd(
        nc, in_maps, core_ids=list(range(NCORES)))

    out = np.empty((B, S, D), np.float32)
    for core in range(NCORES):
        b, rows, mq = rows_per_core[core]
        r = res.results[core]
        o = r["outT"].T  # [1024 q, 1024 dm]
        denf = r["den"].T.reshape(-1)  # [1024] strip-ordered
        denf = np.where(mq, denf, 1.0)  # masked queries: out = v[q] directly
        out[b, rows] = o / denf[:, None]
    return out


# revision 13
# speedup vs baseline: 1.5672x; 1.0127x over previous
"""Trainium2 Bass kernel for nn_Attention_42288247996512 (sparse causal cross-attention).

reference:
  q = x @ Wq.T; k = cross @ Wk.T; v = x @ Wv.T
  logits = q @ k.T  (causal mask; padding mask m_q*m_k + eye > 0)
  out = softmax(logits / sqrt(128)) @ v

Sharding: 8 cores = 4 batches x 2 query-strips (SPMD). Each strip is 8 query
blocks (128 rows) grouped into 4 pairs of adjacent blocks.

Two structural optimizations vs a vanilla flash-style kernel:

1) Reassociation:  attn @ (x @ Wv.T) == (attn @ x) @ Wv.T.  Each core owns
   1024 query rows but would need all 2048 key rows of v, so projecting
   t = attn@x (1024 cols) instead of v (2048 rows) halves that matmul.

2) Key compaction: ~half the keys are padding-masked (exp == 0 columns).
   The kernel is JIT-specialized on the mask's *structure*: keys are
   host-compacted to the active ones, shrinking kT/logits/exp/transpose/AX
   nearly 2x.  Masked queries (whose softmax row is a delta at the diagonal,
   so out[q] = v[q]) bypass attention entirely: x.T is DMA'd into the t
   buffer and the attention results are merged over it with predicated
   copies (mask = query-unmasked), then t @ Wv.T produces v[q] for them
   directly.  The structure parameters are recomputed from the input mask on
   every call (and cached), so the kernel stays correct for any input.

All streamed operands are bf16; PSUM accumulation is f32.  Host does layout
packs/bf16 casts, gathers, additive-mask building, and the final denominator
divide + scatter (as in the baseline kernel).
"""
import math
import threading

import ml_dtypes
import numpy as np

B, S, D, DA = 4, 2048, 1024, 128
P = 128
NCORES = 8
BIG = 32768.0  # power of two: exactly representable in bf16
NQ = 1024      # query rows per core strip
KC = D // P    # 8 contraction chunks of 128

# strips: pairs of adjacent blocks; block g attends orig keys < (g+1)*128
STRIPS = [
    [0, 1, 14, 15, 6, 7, 8, 9],
    [2, 3, 12, 13, 4, 5, 10, 11],
]

_BUILD_LOCK = threading.Lock()
_CACHE: dict = {}


def _derive_params(mask_f):
    """Compute the SPMD kernel structure (max over all 8 cores) from the mask.

    Returns a hashable params tuple:
      nkb:   compacted key blocks (128 each)
      kb:    per-pair key-block count (AX contraction length)
      chunks: per-pair tuple of chunk widths (<=512, multiples of 128)
      masked: per-pair tuple of bools - does chunk j need an additive mask
    """
    nkb = 0
    kb = [0, 0, 0, 0]
    for b in range(B):
        m = mask_f[b] > 0
        ck = np.cumsum(m)          # ck[s] = # active keys <= s
        nk = int(ck[-1])
        nkb = max(nkb, (nk + P - 1) // P)
        for p in range(2):
            blocks = STRIPS[p]
            for pr in range(4):
                g = max(blocks[2 * pr], blocks[2 * pr + 1])
                bmax = int(ck[(g + 1) * P - 1])
                kb[pr] = max(kb[pr], (bmax + P - 1) // P)
    chunks = []
    for pr in range(4):
        w = kb[pr] * P
        ch = []
        while w > 0:
            ch.append(min(512, w))
            w -= min(512, w)
        chunks.append(tuple(ch))
    # chunk (pr, j) needs a mask iff for ANY core its key range reaches
    # beyond that core's (min unmasked-row boundary) or active-key count
    masked = [[False] * len(chunks[pr]) for pr in range(4)]
    for b in range(B):
        m = mask_f[b] > 0
        ck = np.cumsum(m)
        nk = int(ck[-1])
        for p in range(2):
            blocks = STRIPS[p]
            for pr in range(4):
                rows = np.concatenate(
                    [np.arange(g * P, (g + 1) * P)
                     for g in (blocks[2 * pr], blocks[2 * pr + 1])])
                urows = rows[m[rows]]
                bmin = int(ck[urows].min()) if len(urows) else 0
                base = 0
                for j, w in enumerate(chunks[pr]):
                    if base + w > bmin or base + w > nk:
                        masked[pr][j] = True
                    base += w
    return (nkb, tuple(kb), tuple(chunks),
            tuple(tuple(mj) for mj in masked))


def _build(params):
    from contextlib import ExitStack

    import concourse.bass as bass
    import concourse.mybir as mybir
    import concourse.tile as tile
    from concourse import bacc
    from concourse.masks import make_identity

    nkb, KB, CHUNKS, MASKED = params
    NK = max(nkb, 1) * P  # padded compacted key width
    nt = sum(2 * sum(mj) for mj in MASKED)  # dmask tile count
    NT = max(nt, 1)

    dt = mybir.dt
    f32 = dt.float32
    bf16 = dt.bfloat16
    AF = mybir.ActivationFunctionType
    ALU = mybir.AluOpType

    nc = bacc.Bacc("TRN2", target_bir_lowering=False, debug=False)

    # DRAM inputs (bf16 unless noted); weights host-packed to [P, ...] so
    # DMA rows are contiguous >=512B runs.
    xkc = nc.dram_tensor("xkc", [NK, D], bf16, kind="ExternalInput").ap()
    cTc = nc.dram_tensor("cTc", [D, NK], bf16, kind="ExternalInput").ap()
    xqT = nc.dram_tensor("xqT", [D, NQ], bf16, kind="ExternalInput").ap()
    wqp = nc.dram_tensor("wqp", [P, KC, DA], bf16, kind="ExternalInput").ap()
    wkp = nc.dram_tensor("wkp", [P, KC, DA], bf16, kind="ExternalInput").ap()
    wvp = nc.dram_tensor("wvp", [P, KC, D], bf16, kind="ExternalInput").ap()
    qmn = nc.dram_tensor("qmn", [P, 8], f32, kind="ExternalInput").ap()
    dm2 = nc.dram_tensor("dm2", [NT, P, 512], bf16, kind="ExternalInput").ap()
    u8 = dt.uint8
    pmask = nc.dram_tensor("pmask", [P, 4, 256], u8,
                           kind="ExternalInput").ap()

    outT = nc.dram_tensor("outT", [D, NQ], f32, kind="ExternalOutput").ap()
    den = nc.dram_tensor("den", [P, 8], f32, kind="ExternalOutput").ap()

    xkc_r = xkc.rearrange("(kb p) d -> p kb d", p=P)
    cTc_r = cTc.rearrange("(kc p) s -> p kc s", p=P)
    xqT_r = xqT.rearrange("(kc p) q -> p kc q", p=P)
    outT_r = outT.rearrange("(dmc p) q -> p dmc q", p=P)

    # dmask tile index for (pair, chunk, blk): assigned lazily in emission
    # order so the DMA (split in two) streams tiles in first-use order; the
    # host builds dm2 in this same order (read back via _CACHE["dm_order"]).
    dmidx = {}

    def dm_tile(pr, j, blk):
        key = (pr, j, blk)
        if key not in dmidx:
            dmidx[key] = len(dmidx)
        return dmidx[key]

    # kT chunk layout over NK cols
    kt_chunks = []
    w = NK
    while w > 0:
        kt_chunks.append(min(512, w))
        w -= min(512, w)

    with tile.TileContext(nc) as tc, ExitStack() as ctx:
        const = ctx.enter_context(tc.tile_pool(name="const", bufs=1))
        persist = ctx.enter_context(tc.tile_pool(name="persist", bufs=1))
        stream = ctx.enter_context(tc.tile_pool(name="stream", bufs=2))
        apool = ctx.enter_context(tc.tile_pool(name="apool", bufs=4))
        epool = ctx.enter_context(tc.tile_pool(name="epool", bufs=24))

        ident_f32 = const.tile([P, P], f32, name="ident_f32")
        make_identity(nc, ident_f32)
        ident = const.tile([P, P], bf16, name="ident")
        nc.vector.tensor_copy(ident[:], ident_f32[:])

        wq_sb = const.tile([P, KC, DA], bf16, name="wq_sb")
        wk_sb = const.tile([P, KC, DA], bf16, name="wk_sb")
        wv_sb = const.tile([P, KC, D], bf16, name="wv_sb")
        qmn_sb = const.tile([P, 8], f32, name="qmn_sb")
        dm_sb = const.tile([P, NT, 512], bf16, name="dm_sb")
        pm_sb = const.tile([P, 4, 256], u8, name="pm_sb")

        kT_sb = persist.tile([P, NK], bf16, name="kT_sb")
        qT_sb = persist.tile([P, NQ], bf16, name="qT_sb")
        xk_sb = persist.tile([P, max(nkb, 1), D], bf16, name="xk_sb")
        tT_sb = persist.tile([P, KC, NQ], bf16, name="tT_sb")
        den_sb = persist.tile([P, 8], f32, name="den_sb")

        eTs_all = {pr: [] for pr in range(4)}
        daccs_all = {pr: [[], []] for pr in range(4)}

        # PSUM: 4 pools x 2 bufs x 2KB = all 8 banks.
        psl_pool = ctx.enter_context(
            tc.tile_pool(name="psl", bufs=2, space="PSUM"))
        psT_pool = ctx.enter_context(
            tc.tile_pool(name="psT", bufs=2, space="PSUM"))
        psax_pool = ctx.enter_context(
            tc.tile_pool(name="psax", bufs=2, space="PSUM"))
        psw_pool = ctx.enter_context(
            tc.tile_pool(name="psw", bufs=2, space="PSUM"))

        # ---- projections (DMA emission order == SP FIFO delivery order) ----
        def kT_chunk(j):
            if nkb == 0:
                return
            w = kt_chunks[j]
            base = sum(kt_chunks[:j])
            ctj = stream.tile([P, KC, 512], bf16, tag="ct", name=f"ct{j}",
                              bufs=2)
            nc.sync.dma_start(ctj[:, :, :w], cTc_r[:, :, base:base + w])
            ps_k = psax_pool.tile([P, 512], f32, tag="psax", name=f"ps_k{j}")
            for kc in range(KC):
                nc.tensor.matmul(
                    ps_k[:, :w],
                    lhsT=wk_sb[:, kc, :],
                    rhs=ctj[:, kc, :w],
                    start=(kc == 0), stop=(kc == KC - 1),
                )
            nc.any.tensor_copy(kT_sb[:, base:base + w], ps_k[:, :w])

        # PE warmup: dependency-free matmuls on the identity keep the PE busy
        # (and ramp its p-state to full clock) while the first input DMAs
        # stream in; each is only ~120ns so real work is barely delayed.
        for wu in range(48):
            pswu = psw_pool.tile([P, P], f32, tag="psw", name=f"pswu{wu}",
                                 padded_shape=[P, 512])
            nc.tensor.matmul(pswu[:], lhsT=ident[:], rhs=ident[:],
                             start=True, stop=True)

        # t.T is pre-filled with x_strip.T: it doubles as the qT projection
        # rhs AND as the pass-through giving masked queries out[q] = v[q]
        # (attention results are merged over it with predicated copies).
        nc.sync.dma_start(wq_sb[:], wqp)
        ps_q = [psl_pool.tile([P, 512], f32, tag="psl", name=f"ps_q{n}")
                for n in range(2)]

        def qT_half(n):
            nc.sync.dma_start(tT_sb[:, :, n * 512:(n + 1) * 512],
                              xqT_r[:, :, n * 512:(n + 1) * 512])
            for kc in range(KC):
                nc.tensor.matmul(
                    ps_q[n][:],
                    lhsT=wq_sb[:, kc, :],
                    rhs=tT_sb[:, kc, n * 512:(n + 1) * 512],
                    start=(kc == 0), stop=(kc == KC - 1),
                )

        qT_half(0)
        nc.sync.dma_start(wk_sb[:], wkp)
        kT_chunk(0)
        qT_half(1)
        for n in range(2):
            nc.any.tensor_copy(qT_sb[:, n * 512:(n + 1) * 512], ps_q[n][:])
        nc.sync.dma_start(qmn_sb[:], qmn[:])
        NTA = min(6, NT)
        nc.sync.dma_start(dm_sb[:, 0:NTA, :],
                          dm2.rearrange("t p w -> p t w")[:, 0:NTA, :])
        nc.sync.dma_start(pm_sb[:], pmask)

        # ---- attention stages ----
        def stage_a_chunk(pr, j, mid_hook=None):
            w = CHUNKS[pr][j]
            base = sum(CHUNKS[pr][:j])
            nks = (w + P - 1) // P
            psTs = [psT_pool.tile([P, 256], bf16, tag="psT",
                                  name=f"psT{pr}_{j}_{ks}",
                                  padded_shape=[P, 1024])
                    for ks in range(nks)]
            es = []
            for blk in range(2):
                slot = pr * 2 + blk
                psl = psl_pool.tile([P, 512], f32, tag="psl",
                                    name=f"psl{slot}_{j}")
                nc.tensor.matmul(
                    psl[:, :w],
                    lhsT=qT_sb[:, slot * P:(slot + 1) * P],
                    rhs=kT_sb[:, base:base + w],
                    start=True, stop=True,
                )
                dac = apool.tile([P, 1], f32, tag="dac",
                                 name=f"dac{slot}_{j}", bufs=12)
                e = apool.tile([P, 512], bf16, tag="e", name=f"e{slot}_{j}")
                if MASKED[pr][j]:
                    sbl = apool.tile([P, 512], f32, tag="sbl",
                                     name=f"sbl{slot}_{j}")
                    nc.vector.tensor_tensor(
                        out=sbl[:, :w], in0=psl[:, :w],
                        in1=dm_sb[:, dm_tile(pr, j, blk), :w], op=ALU.add)
                    src = sbl
                else:
                    src = psl
                nc.scalar.activation(
                    e[:, :w], src[:, :w], AF.Exp,
                    bias=qmn_sb[:, slot:slot + 1], scale=1.0,
                    accum_out=dac[:],
                )
                daccs_all[pr][blk].append(dac)
                es.append(e)
            if mid_hook is not None:
                mid_hook()
            for blk in range(2):
                for ks in range(nks):
                    nc.tensor.transpose(
                        psTs[ks][:, blk * P:(blk + 1) * P],
                        es[blk][:, ks * P:(ks + 1) * P],
                        ident[:],
                    )
            for ks in range(nks):
                eT = epool.tile([P, 256], bf16, tag="eT",
                                name=f"eT{pr}_{j}_{ks}")
                nc.any.tensor_copy(eT[:], psTs[ks][:])
                eTs_all[pr].append(eT)

        def stage_den(pr):
            for blk in range(2):
                slot = pr * 2 + blk
                dl = daccs_all[pr][blk]
                dst = den_sb[:, slot:slot + 1]
                if len(dl) == 1:
                    nc.any.tensor_copy(dst, dl[0][:])
                else:
                    nc.vector.tensor_tensor(
                        out=dst, in0=dl[0][:], in1=dl[1][:], op=ALU.add)
                    for d in dl[2:]:
                        nc.vector.tensor_tensor(
                            out=dst, in0=dst, in1=d[:], op=ALU.add)

        def stage_ax(pr, mid_hook=None):
            # t.T[xd, q] = sum_kb x[kb].T-contraction with eT over the
            # pair's compacted key blocks; merged into the pass-through
            # with a predicated copy (pmask: 1 = query unmasked).
            stage_den(pr)
            eTs = eTs_all[pr]
            for xdc in range(KC):
                psax = psax_pool.tile([P, 256], f32, tag="psax",
                                      name=f"psax{pr}_{xdc}",
                                      padded_shape=[P, 512])
                for kb in range(KB[pr]):
                    nc.tensor.matmul(
                        psax[:],
                        lhsT=xk_sb[:, kb, xdc * P:(xdc + 1) * P],
                        rhs=eTs[kb][:],
                        start=(kb == 0), stop=(kb == KB[pr] - 1),
                    )
                nc.vector.copy_predicated(
                    tT_sb[:, xdc, pr * 256:(pr + 1) * 256],
                    pm_sb[:, pr, :], psax[:])
                if mid_hook is not None and xdc == 3:
                    mid_hook()

        _osbs = {}

        def stage_twv(pr, dm_half=None, den_dma=False):
            # out.T[dm, q] = Wv.T.T @ t.T for this pair's 256 query columns
            halves = [0, 1] if dm_half is None else [dm_half]
            osb = _osbs.setdefault(
                pr, apool.tile([P, KC, 256], f32, tag="osb",
                               name=f"osb{pr}", bufs=2))
            if den_dma:
                nc.sync.dma_start(den[:], den_sb[:])
            fine = den_dma  # last pair: drain per-dmc so the tail is short
            for h in halves:
                for dmc in range(4 * h, 4 * h + 4):
                    psw = psw_pool.tile([P, 256], f32, tag="psw",
                                        name=f"psw{pr}_{dmc}",
                                        padded_shape=[P, 512])
                    for xdc in range(KC):
                        nc.tensor.matmul(
                            psw[:],
                            lhsT=wv_sb[:, xdc, dmc * P:(dmc + 1) * P],
                            rhs=tT_sb[:, xdc, pr * 256:(pr + 1) * 256],
                            start=(xdc == 0), stop=(xdc == KC - 1),
                        )
                    # alternate engines so the final copies drain in parallel
                    if dmc % 2 == 0:
                        nc.vector.tensor_copy(osb[:, dmc, :], psw[:])
                    else:
                        nc.scalar.copy(osb[:, dmc, :], psw[:])
                    if fine and h == 1:
                        nc.sync.dma_start(
                            outT_r[:, dmc:dmc + 1,
                                   pr * 256:(pr + 1) * 256],
                            osb[:, dmc:dmc + 1, :])
                if not (fine and h == 1):
                    nc.sync.dma_start(
                        outT_r[:, 4 * h:4 * h + 4, pr * 256:(pr + 1) * 256],
                        osb[:, 4 * h:4 * h + 4, :])

        def A(pr, j, mid_hook=None):
            if j < len(CHUNKS[pr]) and CHUNKS[pr][j] > 0:
                stage_a_chunk(pr, j, mid_hook=mid_hook)
            elif mid_hook is not None:
                mid_hook()

        def AX(pr, mid_hook=None):
            if KB[pr] > 0:
                stage_ax(pr, mid_hook=mid_hook)
            elif mid_hook is not None:
                mid_hook()

        # ---- fused schedule (PE emission order tuned to DMA arrivals) ----
        nch = [len(CHUNKS[pr]) for pr in range(4)]
        ka = min(KB[0], nkb)
        kbb = min(max(KB[0], KB[2]), nkb)
        A(0, 0)
        # A(1,0) is the mask-free chunk: cheap PE filler while masks stream
        if nch[1] > 0:
            A(1, 0, mid_hook=lambda: kT_chunk(1)
              if len(kt_chunks) > 1 else None)
        if nch[1] > 1:
            A(1, 1)
        if nkb and ka:
            nc.sync.dma_start(xk_sb[:, 0:ka, :], xkc_r[:, 0:ka, :])
        AX(0)
        if NT > NTA:
            nc.sync.dma_start(dm_sb[:, NTA:NT, :],
                              dm2.rearrange("t p w -> p t w")[:, NTA:NT, :])
        A(2, 0, mid_hook=lambda: [kT_chunk(j) for j in
                                  range(2, len(kt_chunks))])
        A(2, 1)
        if nkb and kbb > ka:
            nc.sync.dma_start(xk_sb[:, ka:kbb, :], xkc_r[:, ka:kbb, :])
        AX(2)
        for j in range(2, nch[1]):
            A(1, j)
        if nkb and nkb > kbb:
            nc.sync.dma_start(xk_sb[:, kbb:nkb, :], xkc_r[:, kbb:nkb, :])
        nc.sync.dma_start(wv_sb[:, :, 0:512], wvp[:, :, 0:512])
        A(3, 0)
        AX(1)
        nc.sync.dma_start(wv_sb[:, :, 512:1024], wvp[:, :, 512:1024])
        stage_twv(0, dm_half=0)
        A(3, 1)
        stage_twv(0, dm_half=1)
        for j in range(2, nch[3]):
            A(3, j)
        stage_twv(2)
        AX(3)
        stage_twv(1)
        stage_twv(3, den_dma=True)

    nc.compile()
    return nc, dmidx


def _get_nc(params=None):
    with _BUILD_LOCK:
        if params is None:
            # harness/test introspection path: last-built (or default) kernel
            if "nc" in _CACHE:
                return _CACHE["nc"]
            params = _CACHE.get("params")
            if params is None:
                raise RuntimeError("call kernel() first to JIT the program")
        if _CACHE.get("params") != params or "nc" not in _CACHE:
            _CACHE["params"] = params
            _CACHE["nc"], _CACHE["dm_order"] = _build(params)
        return _CACHE["nc"]


def kernel(x, cross, Wq, Wk, Wv, mask):
    from concourse import bass_utils

    bf = ml_dtypes.bfloat16
    x = np.asarray(x, dtype=np.float32)
    cross = np.asarray(cross, dtype=np.float32)
    scale = 1.0 / math.sqrt(DA)
    mf = np.asarray(mask).astype(np.float32)  # [B, S]

    params = _derive_params(mf)
    nc = _get_nc(params)
    nkb, KB, CHUNKS, MASKED = params
    NK = max(nkb, 1) * P
    NT = max(sum(2 * sum(mj) for mj in MASKED), 1)

    def pack_w(wT, m_cols):
        # [D, m] -> [P, KC, m] with [p, kc, m] = wT[kc*128 + p, m]
        return np.ascontiguousarray(
            wT.reshape(KC, P, m_cols).transpose(1, 0, 2)).astype(bf)

    wqp_h = pack_w((np.asarray(Wq, np.float32) * scale).T, DA)
    wkp_h = pack_w(np.asarray(Wk, np.float32).T, DA)
    wvp_h = pack_w(np.asarray(Wv, np.float32).T, D)

    in_maps = []
    rows_per_core = []
    for core in range(NCORES):
        b, p = divmod(core, 2)
        blocks = STRIPS[p]
        rows = np.concatenate([np.arange(g * P, (g + 1) * P) for g in blocks])
        mb = mf[b] > 0
        ck = np.cumsum(mb)           # active keys <= s
        active = np.nonzero(mb)[0]   # orig idx of compacted keys
        nk = len(active)
        rows_per_core.append((b, rows, mb[rows]))
        # compacted key-side tensors (zero pad to NK)
        xkc_h = np.zeros((NK, D), np.float32)
        xkc_h[:nk] = x[b][active]
        cTc_h = np.zeros((D, NK), np.float32)
        cTc_h[:, :nk] = cross[b].T[:, active]
        mq = mb[rows]
        qmn_h = np.ascontiguousarray(
            (-BIG * (1.0 - mq.astype(np.float32))).reshape(8, P).T)
        # additive causal/pad masks in compacted key coords, per masked chunk
        dm_h = np.full((NT, P, 512), -BIG, np.float32)
        ck_rows = ck[rows]  # allowed-key count per strip row
        for (pr, j, blk), ti in _CACHE["dm_order"].items():
            w = CHUNKS[pr][j]
            base = sum(CHUNKS[pr][:j])
            ckb = ck_rows[(pr * 2 + blk) * P:(pr * 2 + blk + 1) * P]
            kidx = base + np.arange(w)
            dm_h[ti, :, :w] = np.where(
                kidx[None, :] < ckb[:, None], 0.0, -BIG)
        # predication mask: 1 = query unmasked (take AX result)
        pm_h = np.broadcast_to(
            mq.astype(np.float32).reshape(4, 256)[None, :, :], (P, 4, 256))
        in_maps.append({
            "xkc": xkc_h.astype(bf),
            "cTc": cTc_h.astype(bf),
            "xqT": np.ascontiguousarray(x[b][rows].T).astype(bf),
            "wqp": wqp_h,
            "wkp": wkp_h,
            "wvp": wvp_h,
            "qmn": qmn_h,
            "dm2": dm_h.astype(bf),
            "pmask": np.ascontiguousarray(pm_h).astype(np.uint8),
        })

    _CACHE["in_maps"] = in_maps
    res = bass_utils.run_bass_kernel_spmd(
        nc, in_maps, core_ids=list(range(NCORES)))

    out = np.empty((B, S, D), np.float32)
    for core in range(NCORES):
        b, rows, mq = rows_per_core[core]
        r = res.results[core]
        o = r["outT"].T  # [1024 q, 1024 dm]
        denf = r["den"].T.reshape(-1)  # [1024] strip-ordered
        denf = np.where(mq, denf, 1.0)  # masked queries: out = v[q] directly
        out[b, rows] = o / denf[:, None]
    return out


# revision 14
# speedup vs baseline: 1.5898x; 1.0145x over previous
"""Trainium2 Bass kernel for nn_Attention_42288247996512 (sparse causal cross-attention).

reference:
  q = x @ Wq.T; k = cross @ Wk.T; v = x @ Wv.T
  logits = q @ k.T  (causal mask; padding mask m_q*m_k + eye > 0)
  out = softmax(logits / sqrt(128)) @ v

Sharding: 8 cores = 4 batches x 2 query-strips (SPMD). Each strip is 8 query
blocks (128 rows) grouped into 4 pairs of adjacent blocks.

Two structural optimizations vs a vanilla flash-style kernel:

1) Reassociation:  attn @ (x @ Wv.T) == (attn @ x) @ Wv.T.  Each core owns
   1024 query rows but would need all 2048 key rows of v, so projecting
   t = attn@x (1024 cols) instead of v (2048 rows) halves that matmul.

2) Key compaction: ~half the keys are padding-masked (exp == 0 columns).
   The kernel is JIT-specialized on the mask's *structure*: keys are
   host-compacted to the active ones, shrinking kT/logits/exp/transpose/AX
   nearly 2x.  Masked queries (whose softmax row is a delta at the diagonal,
   so out[q] = v[q]) bypass attention entirely: x.T is DMA'd into the t
   buffer and the attention results are merged over it with predicated
   copies (mask = query-unmasked), then t @ Wv.T produces v[q] for them
   directly.  The structure parameters are recomputed from the input mask on
   every call (and cached), so the kernel stays correct for any input.

All streamed operands are bf16; PSUM accumulation is f32.  Host does layout
packs/bf16 casts, gathers, additive-mask building, and the final denominator
divide + scatter (as in the baseline kernel).
"""
import math
import threading

import ml_dtypes
import numpy as np

B, S, D, DA = 4, 2048, 1024, 128
P = 128
NCORES = 8
BIG = 32768.0  # power of two: exactly representable in bf16
NQ = 1024      # query rows per core strip
KC = D // P    # 8 contraction chunks of 128

# strips: pairs of adjacent blocks; block g attends orig keys < (g+1)*128
STRIPS = [
    [0, 1, 14, 15, 6, 7, 8, 9],
    [2, 3, 12, 13, 4, 5, 10, 11],
]

_BUILD_LOCK = threading.Lock()
_CACHE: dict = {}


def _derive_params(mask_f):
    """Compute the SPMD kernel structure (max over all 8 cores) from the mask.

    Returns a hashable params tuple:
      nkb:   compacted key blocks (128 each)
      kb:    per-pair key-block count (AX contraction length)
      chunks: per-pair tuple of chunk widths (<=512, multiples of 128)
      masked: per-pair tuple of bools - does chunk j need an additive mask
    """
    nkb = 0
    kb = [0, 0, 0, 0]
    for b in range(B):
        m = mask_f[b] > 0
        ck = np.cumsum(m)          # ck[s] = # active keys <= s
        nk = int(ck[-1])
        nkb = max(nkb, (nk + P - 1) // P)
        for p in range(2):
            blocks = STRIPS[p]
            for pr in range(4):
                g = max(blocks[2 * pr], blocks[2 * pr + 1])
                bmax = int(ck[(g + 1) * P - 1])
                kb[pr] = max(kb[pr], (bmax + P - 1) // P)
    chunks = []
    for pr in range(4):
        w = kb[pr] * P
        ch = []
        while w > 0:
            ch.append(min(512, w))
            w -= min(512, w)
        chunks.append(tuple(ch))
    # chunk (pr, j) needs a mask iff for ANY core its key range reaches
    # beyond that core's (min unmasked-row boundary) or active-key count
    masked = [[False] * len(chunks[pr]) for pr in range(4)]
    for b in range(B):
        m = mask_f[b] > 0
        ck = np.cumsum(m)
        nk = int(ck[-1])
        for p in range(2):
            blocks = STRIPS[p]
            for pr in range(4):
                rows = np.concatenate(
                    [np.arange(g * P, (g + 1) * P)
                     for g in (blocks[2 * pr], blocks[2 * pr + 1])])
                urows = rows[m[rows]]
                bmin = int(ck[urows].min()) if len(urows) else 0
                base = 0
                for j, w in enumerate(chunks[pr]):
                    if base + w > bmin or base + w > nk:
                        masked[pr][j] = True
                    base += w
    return (nkb, tuple(kb), tuple(chunks),
            tuple(tuple(mj) for mj in masked))


def _build(params):
    from contextlib import ExitStack

    import concourse.bass as bass
    import concourse.mybir as mybir
    import concourse.tile as tile
    from concourse import bacc
    from concourse.masks import make_identity

    nkb, KB, CHUNKS, MASKED = params
    NK = max(nkb, 1) * P  # padded compacted key width
    nt = sum(2 * sum(mj) for mj in MASKED)  # dmask tile count
    NT = max(nt, 1)

    dt = mybir.dt
    f32 = dt.float32
    bf16 = dt.bfloat16
    AF = mybir.ActivationFunctionType
    ALU = mybir.AluOpType

    nc = bacc.Bacc("TRN2", target_bir_lowering=False, debug=False)

    # DRAM inputs (bf16 unless noted); weights host-packed to [P, ...] so
    # DMA rows are contiguous >=512B runs.
    xkc = nc.dram_tensor("xkc", [NK, D], bf16, kind="ExternalInput").ap()
    cTc = nc.dram_tensor("cTc", [D, NK], bf16, kind="ExternalInput").ap()
    xqT = nc.dram_tensor("xqT", [D, NQ], bf16, kind="ExternalInput").ap()
    wqp = nc.dram_tensor("wqp", [P, KC, DA], bf16, kind="ExternalInput").ap()
    wkp = nc.dram_tensor("wkp", [P, KC, DA], bf16, kind="ExternalInput").ap()
    wvp = nc.dram_tensor("wvp", [P, KC, D], bf16, kind="ExternalInput").ap()
    qmn = nc.dram_tensor("qmn", [P, 8], f32, kind="ExternalInput").ap()
    dm2 = nc.dram_tensor("dm2", [NT, P, 512], bf16, kind="ExternalInput").ap()
    u8 = dt.uint8
    pmask = nc.dram_tensor("pmask", [P, 4, 256], u8,
                           kind="ExternalInput").ap()

    outT = nc.dram_tensor("outT", [D, NQ], f32, kind="ExternalOutput").ap()
    den = nc.dram_tensor("den", [P, 8], f32, kind="ExternalOutput").ap()

    xkc_r = xkc.rearrange("(kb p) d -> p kb d", p=P)
    cTc_r = cTc.rearrange("(kc p) s -> p kc s", p=P)
    xqT_r = xqT.rearrange("(kc p) q -> p kc q", p=P)
    outT_r = outT.rearrange("(dmc p) q -> p dmc q", p=P)

    # dmask tile index for (pair, chunk, blk): assigned lazily in emission
    # order so the DMA (split in two) streams tiles in first-use order; the
    # host builds dm2 in this same order (read back via _CACHE["dm_order"]).
    dmidx = {}

    def dm_tile(pr, j, blk):
        key = (pr, j, blk)
        if key not in dmidx:
            dmidx[key] = len(dmidx)
        return dmidx[key]

    # kT chunk layout over NK cols
    kt_chunks = []
    w = NK
    while w > 0:
        kt_chunks.append(min(512, w))
        w -= min(512, w)

    with tile.TileContext(nc) as tc, ExitStack() as ctx:
        const = ctx.enter_context(tc.tile_pool(name="const", bufs=1))
        persist = ctx.enter_context(tc.tile_pool(name="persist", bufs=1))
        stream = ctx.enter_context(tc.tile_pool(name="stream", bufs=2))
        apool = ctx.enter_context(tc.tile_pool(name="apool", bufs=4))
        epool = ctx.enter_context(tc.tile_pool(name="epool", bufs=24))

        ident_f32 = const.tile([P, P], f32, name="ident_f32")
        make_identity(nc, ident_f32)
        ident = const.tile([P, P], bf16, name="ident")
        nc.vector.tensor_copy(ident[:], ident_f32[:])

        wq_sb = const.tile([P, KC, DA], bf16, name="wq_sb")
        wk_sb = const.tile([P, KC, DA], bf16, name="wk_sb")
        wv_sb = const.tile([P, KC, D], bf16, name="wv_sb")
        qmn_sb = const.tile([P, 8], f32, name="qmn_sb")
        dm_sb = const.tile([P, NT, 512], bf16, name="dm_sb")
        pm_sb = const.tile([P, 4, 256], u8, name="pm_sb")

        kT_sb = persist.tile([P, NK], bf16, name="kT_sb")
        qT_sb = persist.tile([P, NQ], bf16, name="qT_sb")
        xk_sb = persist.tile([P, max(nkb, 1), D], bf16, name="xk_sb")
        tT_sb = persist.tile([P, KC, NQ], bf16, name="tT_sb")
        den_sb = persist.tile([P, 8], f32, name="den_sb")

        eTs_all = {pr: [] for pr in range(4)}
        daccs_all = {pr: [[], []] for pr in range(4)}

        # PSUM: 4 pools x 2 bufs x 2KB = all 8 banks.
        psl_pool = ctx.enter_context(
            tc.tile_pool(name="psl", bufs=2, space="PSUM"))
        psT_pool = ctx.enter_context(
            tc.tile_pool(name="psT", bufs=2, space="PSUM"))
        psax_pool = ctx.enter_context(
            tc.tile_pool(name="psax", bufs=2, space="PSUM"))
        psw_pool = ctx.enter_context(
            tc.tile_pool(name="psw", bufs=2, space="PSUM"))

        # ---- projections (DMA emission order == SP FIFO delivery order) ----
        def kT_chunk(j):
            if nkb == 0:
                return
            w = kt_chunks[j]
            base = sum(kt_chunks[:j])
            ctj = stream.tile([P, KC, 512], bf16, tag="ct", name=f"ct{j}",
                              bufs=2)
            nc.sync.dma_start(ctj[:, :, :w], cTc_r[:, :, base:base + w])
            ps_k = psax_pool.tile([P, 512], f32, tag="psax", name=f"ps_k{j}")
            for kc in range(KC):
                nc.tensor.matmul(
                    ps_k[:, :w],
                    lhsT=wk_sb[:, kc, :],
                    rhs=ctj[:, kc, :w],
                    start=(kc == 0), stop=(kc == KC - 1),
                )
            nc.any.tensor_copy(kT_sb[:, base:base + w], ps_k[:, :w])

        # PE warmup: dependency-free matmuls on the identity keep the PE busy
        # (and ramp its p-state to full clock) while the first input DMAs
        # stream in; each is only ~120ns so real work is barely delayed.
        for wu in range(48):
            pswu = psw_pool.tile([P, P], f32, tag="psw", name=f"pswu{wu}",
                                 padded_shape=[P, 512])
            nc.tensor.matmul(pswu[:], lhsT=ident[:], rhs=ident[:],
                             start=True, stop=True)

        # t.T is pre-filled with x_strip.T: it doubles as the qT projection
        # rhs AND as the pass-through giving masked queries out[q] = v[q]
        # (attention results are merged over it with predicated copies).
        nc.sync.dma_start(wq_sb[:], wqp)
        ps_q = [psl_pool.tile([P, 512], f32, tag="psl", name=f"ps_q{n}")
                for n in range(2)]

        def qT_half(n):
            nc.sync.dma_start(tT_sb[:, :, n * 512:(n + 1) * 512],
                              xqT_r[:, :, n * 512:(n + 1) * 512])
            for kc in range(KC):
                nc.tensor.matmul(
                    ps_q[n][:],
                    lhsT=wq_sb[:, kc, :],
                    rhs=tT_sb[:, kc, n * 512:(n + 1) * 512],
                    start=(kc == 0), stop=(kc == KC - 1),
                )

        qT_half(0)
        nc.sync.dma_start(wk_sb[:], wkp)
        kT_chunk(0)
        qT_half(1)
        for n in range(2):
            nc.any.tensor_copy(qT_sb[:, n * 512:(n + 1) * 512], ps_q[n][:])
        nc.sync.dma_start(qmn_sb[:], qmn[:])
        NTA = min(6, NT)
        nc.sync.dma_start(dm_sb[:, 0:NTA, :],
                          dm2.rearrange("t p w -> p t w")[:, 0:NTA, :])
        nc.sync.dma_start(pm_sb[:], pmask)

        # ---- attention stages ----
        def stage_a_chunk(pr, j, mid_hook=None):
            w = CHUNKS[pr][j]
            base = sum(CHUNKS[pr][:j])
            nks = (w + P - 1) // P
            psTs = [psT_pool.tile([P, 256], bf16, tag="psT",
                                  name=f"psT{pr}_{j}_{ks}",
                                  padded_shape=[P, 1024])
                    for ks in range(nks)]
            es = []
            for blk in range(2):
                slot = pr * 2 + blk
                psl = psl_pool.tile([P, 512], f32, tag="psl",
                                    name=f"psl{slot}_{j}")
                nc.tensor.matmul(
                    psl[:, :w],
                    lhsT=qT_sb[:, slot * P:(slot + 1) * P],
                    rhs=kT_sb[:, base:base + w],
                    start=True, stop=True,
                )
                dac = apool.tile([P, 1], f32, tag="dac",
                                 name=f"dac{slot}_{j}", bufs=12)
                e = apool.tile([P, 512], bf16, tag="e", name=f"e{slot}_{j}")
                if MASKED[pr][j]:
                    sbl = apool.tile([P, 512], f32, tag="sbl",
                                     name=f"sbl{slot}_{j}")
                    nc.vector.tensor_tensor(
                        out=sbl[:, :w], in0=psl[:, :w],
                        in1=dm_sb[:, dm_tile(pr, j, blk), :w], op=ALU.add)
                    src = sbl
                else:
                    src = psl
                nc.scalar.activation(
                    e[:, :w], src[:, :w], AF.Exp,
                    bias=qmn_sb[:, slot:slot + 1], scale=1.0,
                    accum_out=dac[:],
                )
                daccs_all[pr][blk].append(dac)
                es.append(e)
            if mid_hook is not None:
                mid_hook()
            for blk in range(2):
                for ks in range(nks):
                    nc.tensor.transpose(
                        psTs[ks][:, blk * P:(blk + 1) * P],
                        es[blk][:, ks * P:(ks + 1) * P],
                        ident[:],
                    )
            for ks in range(nks):
                eT = epool.tile([P, 256], bf16, tag="eT",
                                name=f"eT{pr}_{j}_{ks}")
                nc.any.tensor_copy(eT[:], psTs[ks][:])
                eTs_all[pr].append(eT)

        def stage_den(pr):
            for blk in range(2):
                slot = pr * 2 + blk
                dl = daccs_all[pr][blk]
                dst = den_sb[:, slot:slot + 1]
                if len(dl) == 1:
                    nc.any.tensor_copy(dst, dl[0][:])
                else:
                    nc.vector.tensor_tensor(
                        out=dst, in0=dl[0][:], in1=dl[1][:], op=ALU.add)
                    for d in dl[2:]:
                        nc.vector.tensor_tensor(
                            out=dst, in0=dst, in1=d[:], op=ALU.add)

        def stage_ax(pr, mid_hook=None):
            # t.T[xd, q] = sum_kb x[kb].T-contraction with eT over the
            # pair's compacted key blocks; merged into the pass-through
            # with a predicated copy (pmask: 1 = query unmasked).
            stage_den(pr)
            eTs = eTs_all[pr]
            for xdc in range(KC):
                psax = psax_pool.tile([P, 256], f32, tag="psax",
                                      name=f"psax{pr}_{xdc}",
                                      padded_shape=[P, 512])
                for kb in range(KB[pr]):
                    nc.tensor.matmul(
                        psax[:],
                        lhsT=xk_sb[:, kb, xdc * P:(xdc + 1) * P],
                        rhs=eTs[kb][:],
                        start=(kb == 0), stop=(kb == KB[pr] - 1),
                    )
                nc.vector.copy_predicated(
                    tT_sb[:, xdc, pr * 256:(pr + 1) * 256],
                    pm_sb[:, pr, :], psax[:])
                if mid_hook is not None and xdc == 3:
                    mid_hook()

        _osbs = {}

        def stage_twv(pr, dm_half=None, den_dma=False):
            # out.T[dm, q] = Wv.T.T @ t.T for this pair's 256 query columns
            halves = [0, 1] if dm_half is None else [dm_half]
            osb = _osbs.setdefault(
                pr, apool.tile([P, KC, 256], f32, tag="osb",
                               name=f"osb{pr}", bufs=2))
            if den_dma:
                nc.scalar.dma_start(den[:], den_sb[:])
            fine = den_dma  # last pair: drain per-dmc so the tail is short
            for h in halves:
                for dmc in range(4 * h, 4 * h + 4):
                    psw = psw_pool.tile([P, 256], f32, tag="psw",
                                        name=f"psw{pr}_{dmc}",
                                        padded_shape=[P, 512])
                    for xdc in range(KC):
                        nc.tensor.matmul(
                            psw[:],
                            lhsT=wv_sb[:, xdc, dmc * P:(dmc + 1) * P],
                            rhs=tT_sb[:, xdc, pr * 256:(pr + 1) * 256],
                            start=(xdc == 0), stop=(xdc == KC - 1),
                        )
                    # alternate engines so the final copies drain in parallel
                    if dmc % 2 == 0:
                        nc.vector.tensor_copy(osb[:, dmc, :], psw[:])
                    else:
                        nc.scalar.copy(osb[:, dmc, :], psw[:])
                    if fine and h == 1:
                        nc.scalar.dma_start(
                            outT_r[:, dmc:dmc + 1,
                                   pr * 256:(pr + 1) * 256],
                            osb[:, dmc:dmc + 1, :])
                if not (fine and h == 1):
                    nc.scalar.dma_start(
                        outT_r[:, 4 * h:4 * h + 4, pr * 256:(pr + 1) * 256],
                        osb[:, 4 * h:4 * h + 4, :])

        def A(pr, j, mid_hook=None):
            if j < len(CHUNKS[pr]) and CHUNKS[pr][j] > 0:
                stage_a_chunk(pr, j, mid_hook=mid_hook)
            elif mid_hook is not None:
                mid_hook()

        def AX(pr, mid_hook=None):
            if KB[pr] > 0:
                stage_ax(pr, mid_hook=mid_hook)
            elif mid_hook is not None:
                mid_hook()

        # ---- fused schedule (PE emission order tuned to DMA arrivals) ----
        nch = [len(CHUNKS[pr]) for pr in range(4)]
        ka = min(KB[0], nkb)
        kbb = min(max(KB[0], KB[2]), nkb)
        A(0, 0)
        # A(1,0) is the mask-free chunk: cheap PE filler while masks stream
        if nch[1] > 0:
            A(1, 0, mid_hook=lambda: kT_chunk(1)
              if len(kt_chunks) > 1 else None)
        if nch[1] > 1:
            A(1, 1)
        if nkb and ka:
            nc.sync.dma_start(xk_sb[:, 0:ka, :], xkc_r[:, 0:ka, :])
        AX(0)
        nc.sync.dma_start(wv_sb[:, :, 0:512], wvp[:, :, 0:512])
        if NT > NTA:
            nc.sync.dma_start(dm_sb[:, NTA:NT, :],
                              dm2.rearrange("t p w -> p t w")[:, NTA:NT, :])
        stage_twv(0, dm_half=0)
        A(2, 0, mid_hook=lambda: [kT_chunk(j) for j in
                                  range(2, len(kt_chunks))])
        A(2, 1)
        if nkb and kbb > ka:
            nc.sync.dma_start(xk_sb[:, ka:kbb, :], xkc_r[:, ka:kbb, :])
        AX(2)
        for j in range(2, nch[1]):
            A(1, j)
        if nkb and nkb > kbb:
            nc.sync.dma_start(xk_sb[:, kbb:nkb, :], xkc_r[:, kbb:nkb, :])
        nc.sync.dma_start(wv_sb[:, :, 512:1024], wvp[:, :, 512:1024])
        A(3, 0)
        AX(1)
        stage_twv(0, dm_half=1)
        stage_twv(2, dm_half=0)
        A(3, 1)
        stage_twv(2, dm_half=1)
        for j in range(2, nch[3]):
            A(3, j)
        AX(3)
        stage_twv(1)
        stage_twv(3, den_dma=True)

    nc.compile()
    return nc, dmidx


def _get_nc(params=None):
    with _BUILD_LOCK:
        if params is None:
            # harness/test introspection path: last-built (or default) kernel
            if "nc" in _CACHE:
                return _CACHE["nc"]
            params = _CACHE.get("params")
            if params is None:
                raise RuntimeError("call kernel() first to JIT the program")
        if _CACHE.get("params") != params or "nc" not in _CACHE:
            _CACHE["params"] = params
            _CACHE["nc"], _CACHE["dm_order"] = _build(params)
        return _CACHE["nc"]


def kernel(x, cross, Wq, Wk, Wv, mask):
    from concourse import bass_utils

    bf = ml_dtypes.bfloat16
    x = np.asarray(x, dtype=np.float32)
    cross = np.asarray(cross, dtype=np.float32)
    scale = 1.0 / math.sqrt(DA)
    mf = np.asarray(mask).astype(np.float32)  # [B, S]

    params = _derive_params(mf)
    nc = _get_nc(params)
    nkb, KB, CHUNKS, MASKED = params
    NK = max(nkb, 1) * P
    NT = max(sum(2 * sum(mj) for mj in MASKED), 1)

    def pack_w(wT, m_cols):
        # [D, m] -> [P, KC, m] with [p, kc, m] = wT[kc*128 + p, m]
        return np.ascontiguousarray(
            wT.reshape(KC, P, m_cols).transpose(1, 0, 2)).astype(bf)

    wqp_h = pack_w((np.asarray(Wq, np.float32) * scale).T, DA)
    wkp_h = pack_w(np.asarray(Wk, np.float32).T, DA)
    wvp_h = pack_w(np.asarray(Wv, np.float32).T, D)

    in_maps = []
    rows_per_core = []
    for core in range(NCORES):
        b, p = divmod(core, 2)
        blocks = STRIPS[p]
        rows = np.concatenate([np.arange(g * P, (g + 1) * P) for g in blocks])
        mb = mf[b] > 0
        ck = np.cumsum(mb)           # active keys <= s
        active = np.nonzero(mb)[0]   # orig idx of compacted keys
        nk = len(active)
        rows_per_core.append((b, rows, mb[rows]))
        # compacted key-side tensors (zero pad to NK)
        xkc_h = np.zeros((NK, D), np.float32)
        xkc_h[:nk] = x[b][active]
        cTc_h = np.zeros((D, NK), np.float32)
        cTc_h[:, :nk] = cross[b].T[:, active]
        mq = mb[rows]
        qmn_h = np.ascontiguousarray(
            (-BIG * (1.0 - mq.astype(np.float32))).reshape(8, P).T)
        # additive causal/pad masks in compacted key coords, per masked chunk
        dm_h = np.full((NT, P, 512), -BIG, np.float32)
        ck_rows = ck[rows]  # allowed-key count per strip row
        for (pr, j, blk), ti in _CACHE["dm_order"].items():
            w = CHUNKS[pr][j]
            base = sum(CHUNKS[pr][:j])
            ckb = ck_rows[(pr * 2 + blk) * P:(pr * 2 + blk + 1) * P]
            kidx = base + np.arange(w)
            dm_h[ti, :, :w] = np.where(
                kidx[None, :] < ckb[:, None], 0.0, -BIG)
        # predication mask: 1 = query unmasked (take AX result)
        pm_h = np.broadcast_to(
            mq.astype(np.float32).reshape(4, 256)[None, :, :], (P, 4, 256))
        in_maps.append({
            "xkc": xkc_h.astype(bf),
            "cTc": cTc_h.astype(bf),
            "xqT": np.ascontiguousarray(x[b][rows].T).astype(bf),
            "wqp": wqp_h,
            "wkp": wkp_h,
            "wvp": wvp_h,
            "qmn": qmn_h,
            "dm2": dm_h.astype(bf),
            "pmask": np.ascontiguousarray(pm_h).astype(np.uint8),
        })

    _CACHE["in_maps"] = in_maps
    res = bass_utils.run_bass_kernel_spmd(
        nc, in_maps, core_ids=list(range(NCORES)))

    out = np.empty((B, S, D), np.float32)
    for core in range(NCORES):
        b, rows, mq = rows_per_core[core]
        r = res.results[core]
        o = r["outT"].T  # [1024 q, 1024 dm]
        denf = r["den"].T.reshape(-1)  # [1024] strip-ordered
        denf = np.where(mq, denf, 1.0)  # masked queries: out = v[q] directly
        out[b, rows] = o / denf[:, None]
    return out


# revision 15
# speedup vs baseline: 1.6172x; 1.0172x over previous
"""Trainium2 Bass kernel for nn_Attention_42288247996512 (sparse causal cross-attention).

reference:
  q = x @ Wq.T; k = cross @ Wk.T; v = x @ Wv.T
  logits = q @ k.T  (causal mask; padding mask m_q*m_k + eye > 0)
  out = softmax(logits / sqrt(128)) @ v

Sharding: 8 cores = 4 batches x 2 query-strips (SPMD). Each strip is 8 query
blocks (128 rows) grouped into 4 pairs of adjacent blocks.

Two structural optimizations vs a vanilla flash-style kernel:

1) Reassociation:  attn @ (x @ Wv.T) == (attn @ x) @ Wv.T.  Each core owns
   1024 query rows but would need all 2048 key rows of v, so projecting
   t = attn@x (1024 cols) instead of v (2048 rows) halves that matmul.

2) Key compaction: ~half the keys are padding-masked (exp == 0 columns).
   The kernel is JIT-specialized on the mask's *structure*: keys are
   host-compacted to the active ones, shrinking kT/logits/exp/transpose/AX
   nearly 2x.  Masked queries (whose softmax row is a delta at the diagonal,
   so out[q] = v[q]) bypass attention entirely: x.T is DMA'd into the t
   buffer and the attention results are merged over it with predicated
   copies (mask = query-unmasked), then t @ Wv.T produces v[q] for them
   directly.  The structure parameters are recomputed from the input mask on
   every call (and cached), so the kernel stays correct for any input.

All streamed operands are bf16; PSUM accumulation is f32.  Host does layout
packs/bf16 casts, gathers, additive-mask building, and the final denominator
divide + scatter (as in the baseline kernel).
"""
import math
import threading

import ml_dtypes
import numpy as np

B, S, D, DA = 4, 2048, 1024, 128
P = 128
NCORES = 8
BIG = 32768.0  # power of two: exactly representable in bf16
NQ = 1024      # query rows per core strip
KC = D // P    # 8 contraction chunks of 128

# strips: pairs of adjacent blocks; block g attends orig keys < (g+1)*128
STRIPS = [
    [0, 1, 14, 15, 6, 7, 8, 9],
    [2, 3, 12, 13, 4, 5, 10, 11],
]

_BUILD_LOCK = threading.Lock()
_CACHE: dict = {}


def _derive_params(mask_f):
    """Compute the SPMD kernel structure (max over all 8 cores) from the mask.

    Returns a hashable params tuple:
      nkb:   compacted key blocks (128 each)
      kb:    per-pair key-block count (AX contraction length)
      chunks: per-pair tuple of chunk widths (<=512, multiples of 128)
      masked: per-pair tuple of bools - does chunk j need an additive mask
    """
    nkb = 0
    kb = [0, 0, 0, 0]
    for b in range(B):
        m = mask_f[b] > 0
        ck = np.cumsum(m)          # ck[s] = # active keys <= s
        nk = int(ck[-1])
        nkb = max(nkb, (nk + P - 1) // P)
        for p in range(2):
            blocks = STRIPS[p]
            for pr in range(4):
                g = max(blocks[2 * pr], blocks[2 * pr + 1])
                bmax = int(ck[(g + 1) * P - 1])
                kb[pr] = max(kb[pr], (bmax + P - 1) // P)
    chunks = []
    for pr in range(4):
        w = kb[pr] * P
        ch = []
        while w > 0:
            ch.append(min(512, w))
            w -= min(512, w)
        chunks.append(tuple(ch))
    # chunk (pr, j) needs a mask iff for ANY core its key range reaches
    # beyond that core's (min unmasked-row boundary) or active-key count
    masked = [[False] * len(chunks[pr]) for pr in range(4)]
    for b in range(B):
        m = mask_f[b] > 0
        ck = np.cumsum(m)
        nk = int(ck[-1])
        for p in range(2):
            blocks = STRIPS[p]
            for pr in range(4):
                rows = np.concatenate(
                    [np.arange(g * P, (g + 1) * P)
                     for g in (blocks[2 * pr], blocks[2 * pr + 1])])
                urows = rows[m[rows]]
                bmin = int(ck[urows].min()) if len(urows) else 0
                base = 0
                for j, w in enumerate(chunks[pr]):
                    if base + w > bmin or base + w > nk:
                        masked[pr][j] = True
                    base += w
    return (nkb, tuple(kb), tuple(chunks),
            tuple(tuple(mj) for mj in masked))


def _build(params):
    from contextlib import ExitStack

    import concourse.bass as bass
    import concourse.mybir as mybir
    import concourse.tile as tile
    from concourse import bacc
    from concourse.masks import make_identity

    nkb, KB, CHUNKS, MASKED = params
    NK = max(nkb, 1) * P  # padded compacted key width
    nt = sum(2 * sum(mj) for mj in MASKED)  # dmask tile count
    NT = max(nt, 1)

    dt = mybir.dt
    f32 = dt.float32
    bf16 = dt.bfloat16
    AF = mybir.ActivationFunctionType
    ALU = mybir.AluOpType

    nc = bacc.Bacc("TRN2", target_bir_lowering=False, debug=False)

    # DRAM inputs (bf16 unless noted); weights host-packed to [P, ...] so
    # DMA rows are contiguous >=512B runs.
    xkc = nc.dram_tensor("xkc", [NK, D], bf16, kind="ExternalInput").ap()
    cTc = nc.dram_tensor("cTc", [D, NK], bf16, kind="ExternalInput").ap()
    xqT = nc.dram_tensor("xqT", [D, NQ], bf16, kind="ExternalInput").ap()
    wqp = nc.dram_tensor("wqp", [P, KC, DA], bf16, kind="ExternalInput").ap()
    wkp = nc.dram_tensor("wkp", [P, KC, DA], bf16, kind="ExternalInput").ap()
    wvp = nc.dram_tensor("wvp", [P, KC, D], bf16, kind="ExternalInput").ap()
    qmn = nc.dram_tensor("qmn", [P, 8], f32, kind="ExternalInput").ap()
    dm2 = nc.dram_tensor("dm2", [NT, P, 512], bf16, kind="ExternalInput").ap()
    u8 = dt.uint8
    pmask = nc.dram_tensor("pmask", [P, 4, 256], u8,
                           kind="ExternalInput").ap()

    outT = nc.dram_tensor("outT", [D, NQ], bf16,
                          kind="ExternalOutput").ap()
    den = nc.dram_tensor("den", [P, 8], f32, kind="ExternalOutput").ap()

    xkc_r = xkc.rearrange("(kb p) d -> p kb d", p=P)
    cTc_r = cTc.rearrange("(kc p) s -> p kc s", p=P)
    xqT_r = xqT.rearrange("(kc p) q -> p kc q", p=P)
    outT_r = outT.rearrange("(dmc p) q -> p dmc q", p=P)

    # dmask tile index for (pair, chunk, blk): assigned lazily in emission
    # order so the DMA (split in two) streams tiles in first-use order; the
    # host builds dm2 in this same order (read back via _CACHE["dm_order"]).
    dmidx = {}

    def dm_tile(pr, j, blk):
        key = (pr, j, blk)
        if key not in dmidx:
            dmidx[key] = len(dmidx)
        return dmidx[key]

    # kT chunk layout over NK cols
    kt_chunks = []
    w = NK
    while w > 0:
        kt_chunks.append(min(512, w))
        w -= min(512, w)

    with tile.TileContext(nc) as tc, ExitStack() as ctx:
        const = ctx.enter_context(tc.tile_pool(name="const", bufs=1))
        persist = ctx.enter_context(tc.tile_pool(name="persist", bufs=1))
        stream = ctx.enter_context(tc.tile_pool(name="stream", bufs=2))
        apool = ctx.enter_context(tc.tile_pool(name="apool", bufs=4))
        epool = ctx.enter_context(tc.tile_pool(name="epool", bufs=24))

        ident_f32 = const.tile([P, P], f32, name="ident_f32")
        make_identity(nc, ident_f32)
        ident = const.tile([P, P], bf16, name="ident")
        nc.vector.tensor_copy(ident[:], ident_f32[:])

        wq_sb = const.tile([P, KC, DA], bf16, name="wq_sb")
        wk_sb = const.tile([P, KC, DA], bf16, name="wk_sb")
        wv_sb = const.tile([P, KC, D], bf16, name="wv_sb")
        qmn_sb = const.tile([P, 8], f32, name="qmn_sb")
        dm_sb = const.tile([P, NT, 512], bf16, name="dm_sb")
        pm_sb = const.tile([P, 4, 256], u8, name="pm_sb")

        kT_sb = persist.tile([P, NK], bf16, name="kT_sb")
        qT_sb = persist.tile([P, NQ], bf16, name="qT_sb")
        xk_sb = persist.tile([P, max(nkb, 1), D], bf16, name="xk_sb")
        tT_sb = persist.tile([P, KC, NQ], bf16, name="tT_sb")
        den_sb = persist.tile([P, 8], f32, name="den_sb")

        eTs_all = {pr: [] for pr in range(4)}
        daccs_all = {pr: [[], []] for pr in range(4)}

        # PSUM: 4 pools x 2 bufs x 2KB = all 8 banks.
        psl_pool = ctx.enter_context(
            tc.tile_pool(name="psl", bufs=2, space="PSUM"))
        psT_pool = ctx.enter_context(
            tc.tile_pool(name="psT", bufs=2, space="PSUM"))
        psax_pool = ctx.enter_context(
            tc.tile_pool(name="psax", bufs=2, space="PSUM"))
        psw_pool = ctx.enter_context(
            tc.tile_pool(name="psw", bufs=2, space="PSUM"))

        # ---- projections (DMA emission order == SP FIFO delivery order) ----
        def kT_chunk(j):
            if nkb == 0:
                return
            w = kt_chunks[j]
            base = sum(kt_chunks[:j])
            ctj = stream.tile([P, KC, 512], bf16, tag="ct", name=f"ct{j}",
                              bufs=2)
            nc.sync.dma_start(ctj[:, :, :w], cTc_r[:, :, base:base + w])
            ps_k = psax_pool.tile([P, 512], f32, tag="psax", name=f"ps_k{j}")
            for kc in range(KC):
                nc.tensor.matmul(
                    ps_k[:, :w],
                    lhsT=wk_sb[:, kc, :],
                    rhs=ctj[:, kc, :w],
                    start=(kc == 0), stop=(kc == KC - 1),
                )
            nc.any.tensor_copy(kT_sb[:, base:base + w], ps_k[:, :w])

        # PE warmup: dependency-free matmuls on the identity keep the PE busy
        # (and ramp its p-state to full clock) while the first input DMAs
        # stream in; each is only ~120ns so real work is barely delayed.
        for wu in range(48):
            pswu = psw_pool.tile([P, P], f32, tag="psw", name=f"pswu{wu}",
                                 padded_shape=[P, 512])
            nc.tensor.matmul(pswu[:], lhsT=ident[:], rhs=ident[:],
                             start=True, stop=True)

        # t.T is pre-filled with x_strip.T: it doubles as the qT projection
        # rhs AND as the pass-through giving masked queries out[q] = v[q]
        # (attention results are merged over it with predicated copies).
        nc.sync.dma_start(wq_sb[:], wqp)
        ps_q = [psl_pool.tile([P, 512], f32, tag="psl", name=f"ps_q{n}")
                for n in range(2)]

        def qT_half(n, split=1):
            for s in range(split):
                k0, k1 = s * KC // split, (s + 1) * KC // split
                nc.sync.dma_start(tT_sb[:, k0:k1, n * 512:(n + 1) * 512],
                                  xqT_r[:, k0:k1, n * 512:(n + 1) * 512])
            for kc in range(KC):
                nc.tensor.matmul(
                    ps_q[n][:],
                    lhsT=wq_sb[:, kc, :],
                    rhs=tT_sb[:, kc, n * 512:(n + 1) * 512],
                    start=(kc == 0), stop=(kc == KC - 1),
                )

        qT_half(0)
        nc.sync.dma_start(wk_sb[:], wkp)
        kT_chunk(0)
        qT_half(1, split=2)
        for n in range(2):
            nc.any.tensor_copy(qT_sb[:, n * 512:(n + 1) * 512], ps_q[n][:])
        nc.sync.dma_start(qmn_sb[:], qmn[:])
        NTA = min(6, NT)
        nc.sync.dma_start(dm_sb[:, 0:NTA, :],
                          dm2.rearrange("t p w -> p t w")[:, 0:NTA, :])
        nc.sync.dma_start(pm_sb[:], pmask)

        # ---- attention stages ----
        def stage_a_chunk(pr, j, mid_hook=None):
            w = CHUNKS[pr][j]
            base = sum(CHUNKS[pr][:j])
            nks = (w + P - 1) // P
            psTs = [psT_pool.tile([P, 256], bf16, tag="psT",
                                  name=f"psT{pr}_{j}_{ks}",
                                  padded_shape=[P, 1024])
                    for ks in range(nks)]
            es = []
            for blk in range(2):
                slot = pr * 2 + blk
                psl = psl_pool.tile([P, 512], f32, tag="psl",
                                    name=f"psl{slot}_{j}")
                nc.tensor.matmul(
                    psl[:, :w],
                    lhsT=qT_sb[:, slot * P:(slot + 1) * P],
                    rhs=kT_sb[:, base:base + w],
                    start=True, stop=True,
                )
                dac = apool.tile([P, 1], f32, tag="dac",
                                 name=f"dac{slot}_{j}", bufs=12)
                e = apool.tile([P, 512], bf16, tag="e", name=f"e{slot}_{j}")
                if MASKED[pr][j]:
                    sbl = apool.tile([P, 512], f32, tag="sbl",
                                     name=f"sbl{slot}_{j}")
                    nc.vector.tensor_tensor(
                        out=sbl[:, :w], in0=psl[:, :w],
                        in1=dm_sb[:, dm_tile(pr, j, blk), :w], op=ALU.add)
                    src = sbl
                else:
                    src = psl
                nc.scalar.activation(
                    e[:, :w], src[:, :w], AF.Exp,
                    bias=qmn_sb[:, slot:slot + 1], scale=1.0,
                    accum_out=dac[:],
                )
                daccs_all[pr][blk].append(dac)
                es.append(e)
            if mid_hook is not None:
                mid_hook()
            for blk in range(2):
                for ks in range(nks):
                    nc.tensor.transpose(
                        psTs[ks][:, blk * P:(blk + 1) * P],
                        es[blk][:, ks * P:(ks + 1) * P],
                        ident[:],
                    )
            for ks in range(nks):
                eT = epool.tile([P, 256], bf16, tag="eT",
                                name=f"eT{pr}_{j}_{ks}")
                nc.any.tensor_copy(eT[:], psTs[ks][:])
                eTs_all[pr].append(eT)

        def stage_den(pr):
            for blk in range(2):
                slot = pr * 2 + blk
                dl = daccs_all[pr][blk]
                dst = den_sb[:, slot:slot + 1]
                if len(dl) == 1:
                    nc.any.tensor_copy(dst, dl[0][:])
                else:
                    nc.vector.tensor_tensor(
                        out=dst, in0=dl[0][:], in1=dl[1][:], op=ALU.add)
                    for d in dl[2:]:
                        nc.vector.tensor_tensor(
                            out=dst, in0=dst, in1=d[:], op=ALU.add)

        def stage_ax(pr, mid_hook=None):
            # t.T[xd, q] = sum_kb x[kb].T-contraction with eT over the
            # pair's compacted key blocks; merged into the pass-through
            # with a predicated copy (pmask: 1 = query unmasked).
            stage_den(pr)
            eTs = eTs_all[pr]
            for xdc in range(KC):
                psax = psax_pool.tile([P, 256], f32, tag="psax",
                                      name=f"psax{pr}_{xdc}",
                                      padded_shape=[P, 512])
                for kb in range(KB[pr]):
                    nc.tensor.matmul(
                        psax[:],
                        lhsT=xk_sb[:, kb, xdc * P:(xdc + 1) * P],
                        rhs=eTs[kb][:],
                        start=(kb == 0), stop=(kb == KB[pr] - 1),
                    )
                nc.vector.copy_predicated(
                    tT_sb[:, xdc, pr * 256:(pr + 1) * 256],
                    pm_sb[:, pr, :], psax[:])
                if mid_hook is not None and xdc == 3:
                    mid_hook()

        _osbs = {}

        def stage_twv(pr, dm_half=None, den_dma=False):
            # out.T[dm, q] = Wv.T.T @ t.T for this pair's 256 query columns
            halves = [0, 1] if dm_half is None else [dm_half]
            osb = _osbs.setdefault(
                pr, apool.tile([P, KC, 256], bf16, tag="osb",
                               name=f"osb{pr}", bufs=2))
            if den_dma:
                nc.scalar.dma_start(den[:], den_sb[:])
            fine = den_dma  # last pair: drain per-dmc so the tail is short
            for h in halves:
                for dmc in range(4 * h, 4 * h + 4):
                    psw = psw_pool.tile([P, 256], f32, tag="psw",
                                        name=f"psw{pr}_{dmc}",
                                        padded_shape=[P, 512])
                    for xdc in range(KC):
                        nc.tensor.matmul(
                            psw[:],
                            lhsT=wv_sb[:, xdc, dmc * P:(dmc + 1) * P],
                            rhs=tT_sb[:, xdc, pr * 256:(pr + 1) * 256],
                            start=(xdc == 0), stop=(xdc == KC - 1),
                        )
                    # alternate engines so the final copies drain in parallel
                    if fine or dmc % 2 == 0:
                        nc.vector.tensor_copy(osb[:, dmc, :], psw[:])
                    else:
                        nc.scalar.copy(osb[:, dmc, :], psw[:])
                    if fine and h == 1:
                        nc.sync.dma_start(
                            outT_r[:, dmc:dmc + 1,
                                   pr * 256:(pr + 1) * 256],
                            osb[:, dmc:dmc + 1, :])
                if not (fine and h == 1):
                    nc.scalar.dma_start(
                        outT_r[:, 4 * h:4 * h + 4, pr * 256:(pr + 1) * 256],
                        osb[:, 4 * h:4 * h + 4, :])

        def A(pr, j, mid_hook=None):
            if j < len(CHUNKS[pr]) and CHUNKS[pr][j] > 0:
                stage_a_chunk(pr, j, mid_hook=mid_hook)
            elif mid_hook is not None:
                mid_hook()

        def AX(pr, mid_hook=None):
            if KB[pr] > 0:
                stage_ax(pr, mid_hook=mid_hook)
            elif mid_hook is not None:
                mid_hook()

        # ---- fused schedule (PE emission order tuned to DMA arrivals) ----
        nch = [len(CHUNKS[pr]) for pr in range(4)]
        ka = min(KB[0], nkb)
        kbb = min(max(KB[0], KB[2]), nkb)
        A(0, 0)
        # A(1,0) is the mask-free chunk: cheap PE filler while masks stream
        if nch[1] > 0:
            A(1, 0, mid_hook=lambda: kT_chunk(1)
              if len(kt_chunks) > 1 else None)
        if nch[1] > 1:
            A(1, 1)
        if nkb and ka:
            nc.sync.dma_start(xk_sb[:, 0:ka, :], xkc_r[:, 0:ka, :])
        AX(0)
        nc.sync.dma_start(wv_sb[:, :, 0:512], wvp[:, :, 0:512])
        if NT > NTA:
            nc.sync.dma_start(dm_sb[:, NTA:NT, :],
                              dm2.rearrange("t p w -> p t w")[:, NTA:NT, :])
        stage_twv(0, dm_half=0)
        A(2, 0, mid_hook=lambda: [kT_chunk(j) for j in
                                  range(2, len(kt_chunks))])
        A(2, 1)
        if nkb and kbb > ka:
            nc.sync.dma_start(xk_sb[:, ka:kbb, :], xkc_r[:, ka:kbb, :])
        AX(2)
        for j in range(2, nch[1]):
            A(1, j)
        if nkb and nkb > kbb:
            nc.sync.dma_start(xk_sb[:, kbb:nkb, :], xkc_r[:, kbb:nkb, :])
        nc.sync.dma_start(wv_sb[:, :, 512:1024], wvp[:, :, 512:1024])
        A(3, 0)
        AX(1)
        stage_twv(0, dm_half=1)
        stage_twv(2, dm_half=0)
        A(3, 1)
        stage_twv(2, dm_half=1)
        for j in range(2, nch[3]):
            A(3, j)
        AX(3)
        stage_twv(1)
        stage_twv(3, den_dma=True)

    nc.compile()
    return nc, dmidx


def _get_nc(params=None):
    with _BUILD_LOCK:
        if params is None:
            # harness/test introspection path: last-built (or default) kernel
            if "nc" in _CACHE:
                return _CACHE["nc"]
            params = _CACHE.get("params")
            if params is None:
                raise RuntimeError("call kernel() first to JIT the program")
        if _CACHE.get("params") != params or "nc" not in _CACHE:
            _CACHE["params"] = params
            _CACHE["nc"], _CACHE["dm_order"] = _build(params)
        return _CACHE["nc"]


def kernel(x, cross, Wq, Wk, Wv, mask):
    from concourse import bass_utils

    bf = ml_dtypes.bfloat16
    x = np.asarray(x, dtype=np.float32)
    cross = np.asarray(cross, dtype=np.float32)
    scale = 1.0 / math.sqrt(DA)
    mf = np.asarray(mask).astype(np.float32)  # [B, S]

    params = _derive_params(mf)
    nc = _get_nc(params)
    nkb, KB, CHUNKS, MASKED = params
    NK = max(nkb, 1) * P
    NT = max(sum(2 * sum(mj) for mj in MASKED), 1)

    def pack_w(wT, m_cols):
        # [D, m] -> [P, KC, m] with [p, kc, m] = wT[kc*128 + p, m]
        return np.ascontiguousarray(
            wT.reshape(KC, P, m_cols).transpose(1, 0, 2)).astype(bf)

    wqp_h = pack_w((np.asarray(Wq, np.float32) * scale).T, DA)
    wkp_h = pack_w(np.asarray(Wk, np.float32).T, DA)
    wvp_h = pack_w(np.asarray(Wv, np.float32).T, D)

    in_maps = []
    rows_per_core = []
    for core in range(NCORES):
        b, p = divmod(core, 2)
        blocks = STRIPS[p]
        rows = np.concatenate([np.arange(g * P, (g + 1) * P) for g in blocks])
        mb = mf[b] > 0
        ck = np.cumsum(mb)           # active keys <= s
        active = np.nonzero(mb)[0]   # orig idx of compacted keys
        nk = len(active)
        rows_per_core.append((b, rows, mb[rows]))
        # compacted key-side tensors (zero pad to NK)
        xkc_h = np.zeros((NK, D), np.float32)
        xkc_h[:nk] = x[b][active]
        cTc_h = np.zeros((D, NK), np.float32)
        cTc_h[:, :nk] = cross[b].T[:, active]
        mq = mb[rows]
        qmn_h = np.ascontiguousarray(
            (-BIG * (1.0 - mq.astype(np.float32))).reshape(8, P).T)
        # additive causal/pad masks in compacted key coords, per masked chunk
        dm_h = np.full((NT, P, 512), -BIG, np.float32)
        ck_rows = ck[rows]  # allowed-key count per strip row
        for (pr, j, blk), ti in _CACHE["dm_order"].items():
            w = CHUNKS[pr][j]
            base = sum(CHUNKS[pr][:j])
            ckb = ck_rows[(pr * 2 + blk) * P:(pr * 2 + blk + 1) * P]
            kidx = base + np.arange(w)
            dm_h[ti, :, :w] = np.where(
                kidx[None, :] < ckb[:, None], 0.0, -BIG)
        # predication mask: 1 = query unmasked (take AX result)
        pm_h = np.broadcast_to(
            mq.astype(np.float32).reshape(4, 256)[None, :, :], (P, 4, 256))
        in_maps.append({
            "xkc": xkc_h.astype(bf),
            "cTc": cTc_h.astype(bf),
            "xqT": np.ascontiguousarray(x[b][rows].T).astype(bf),
            "wqp": wqp_h,
            "wkp": wkp_h,
            "wvp": wvp_h,
            "qmn": qmn_h,
            "dm2": dm_h.astype(bf),
            "pmask": np.ascontiguousarray(pm_h).astype(np.uint8),
        })

    _CACHE["in_maps"] = in_maps
    res = bass_utils.run_bass_kernel_spmd(
        nc, in_maps, core_ids=list(range(NCORES)))

    out = np.empty((B, S, D), np.float32)
    for core in range(NCORES):
        b, rows, mq = rows_per_core[core]
        r = res.results[core]
        o = r["outT"].T.astype(np.float32)  # [1024 q, 1024 dm]
        denf = r["den"].T.reshape(-1)  # [1024] strip-ordered
        denf = np.where(mq, denf, 1.0)  # masked queries: out = v[q] directly
        out[b, rows] = o / denf[:, None]
    return out
